# revision 35
# baseline (speedup 1.0000x reference)
"""Multi-headed attention (B=2, L=2048, E=1024, H=16) on 8 trn2 cores.

Sharding: batch (2) x head-groups (4) -> 8 cores. Each core computes 4 heads
of one batch element end-to-end (QKV projection, attention, partial output
projection); host sums the per-head-group partial outputs (out + out2) per
batch and adds the final bias.

All data is fp16 (host converts); PSUM accumulates fp32. Design, driven by
the TimelineSim cost model (matmul cost = out free size; exp is ACT-only):
  - QKV projections: W-slice stationary, x^T moving (N=512 chains).
  - Scores S^T [k, q] per head into [128,1024] PSUM, one wide exp per
    (head, k-tile) on ACT straight to fp16 SBUF (1038ns each; ACT busy
    ~134us is the co-critical path with PE ~140us).
  - PV flipped to out[q, d]: pt (exp'd scores) stationary, V [k, 64+1]
    moving (N=65; ones column accumulates the softmax denominator) -- half
    the PE rows of the N=q orientation.
  - Normalization fused into the ctx eviction (reciprocal + 0-stride
    broadcast multiply, batched per PSUM bank); ctx^T via DMA transpose
    (mid-kernel) or PE transpose (final pass, avoids DMA latency).
  - Attention runs in 4 passes of 2 heads ordered (qcp0,g0),(qcp1,g0),
    (qcp0,g1),(qcp1,g1) so passes 1-2 reuse g0 K/V and all g1 projection
    chains spread into later passes, balancing PE load per pass.
  - PSUM (8 banks): scores 2x[128,1024] (4) + ctx regions 130-wide pairs
    (3) + misc (1); one start/stop per bank per pass (zero-region rule).
  - Emission is software-pipelined: scores/exp run ~2 chunks ahead of PV;
    projection chains weave in as ~3-matmul micro-pieces; out-projection:
    qcp0 whole octiles in pass 4, qcp1 g0-part DMA'd to out2 during pass 3
    (host adds), g1-part at the tail woven with per-lt PE transposes.
"""

import numpy as np

EMBED = 1024
HEADS = 16
HD = 64
B = 2
L = 2048
N_CORES = 8
HPC = 4              # heads per core
ES = HPC * HD        # 256: e-slice width per core
NEC = EMBED // 128   # 8 embed chunks
NKT = L // 128       # 16 k-tiles

_CACHE = {}
EMITLOG = []


def _gen_kernel():
    from contextlib import ExitStack

    import concourse.mybir as mybir
    import concourse.tile as tile
    from concourse import bacc
    from concourse.masks import make_identity

    dt = mybir.dt
    f32 = dt.float32
    f16 = dt.float16

    nc = bacc.Bacc("TRN2", target_bir_lowering=False)

    xqT = nc.dram_tensor("xqT", [EMBED, L], f16, kind="ExternalInput")
    xkT = nc.dram_tensor("xkT", [EMBED, L], f16, kind="ExternalInput")
    xvT = nc.dram_tensor("xvT", [EMBED, L], f16, kind="ExternalInput")
    wT = nc.dram_tensor("wT", [EMBED, ES], f16, kind="ExternalInput")
    woT = nc.dram_tensor("woT", [ES, EMBED], f16, kind="ExternalInput")
    bqkv = nc.dram_tensor("bqkv", [128, 2], f32, kind="ExternalInput")
    out = nc.dram_tensor("out", [L, EMBED], f16, kind="ExternalOutput")
    out2 = nc.dram_tensor("out2", [L // 2, EMBED], f16, kind="ExternalOutput")

    # ctx region offset: (q8, hh) -> free offset in the 3-bank ctx tile.
    # Pairs of 130 (two heads x 65), 3 pairs per 512-f32 bank, no straddle.
    def ctx_off(q8, hh):
        return (q8 // 3) * 512 + (q8 % 3) * 130 + hh * 65

    with tile.TileContext(nc) as tc, ExitStack() as ctx:
        const = ctx.enter_context(tc.tile_pool(name="const", bufs=1))
        xin = ctx.enter_context(tc.tile_pool(name="xin", bufs=1))
        big = ctx.enter_context(tc.tile_pool(name="big", bufs=1))
        work = ctx.enter_context(tc.tile_pool(name="work", bufs=2))
        # PSUM budget (8 banks): scores 2x2 + ctx 3 + misc 1
        pp_s = ctx.enter_context(tc.tile_pool(name="pp_s", bufs=2, space="PSUM"))
        pp_ctx = ctx.enter_context(tc.tile_pool(name="pp_ctx", bufs=1, space="PSUM"))
        pp_m = ctx.enter_context(tc.tile_pool(name="pp_m", bufs=1, space="PSUM"))

        # ---- constants (g0 half of wt first; wo deferred) ------------------
        wt = const.tile([128, NEC * ES], f16)
        wt_v = wt[:].rearrange("p (c g e) -> p c g e", c=NEC, g=2)
        wT_v = wT[:].rearrange("(c p) (g e) -> p c g e", p=128, g=2)
        nc.sync.dma_start(wt_v[:, :, 0, :], wT_v[:, :, 0, :])
        bq = const.tile([128, 2], f32)
        nc.sync.dma_start(bq[:], bqkv[:])

        idn = const.tile([128, 128], f16)
        make_identity(nc, idn[:])

        # ---- x staging (fp16; xq chunk-tiles, xk/xv single tiles) ----------
        xq_sb = [xin.tile([128, L], f16, name=f"xq{c}") for c in range(NEC)]
        xk_sb = xin.tile([128, NEC * L], f16, name="xk")
        xv_sb = xin.tile([128, NEC * L], f16, name="xv")

        def xk_ap(c, qc):
            return xk_sb[:, c * L + qc * 512:(c * L) + (qc + 1) * 512]

        def xv_ap(c, qc):
            return xv_sb[:, c * L + qc * 512:(c * L) + (qc + 1) * 512]

        def big_dma(xsb, xdram, lo, hi):
            nc.sync.dma_start(
                xsb[:].rearrange("p (c l) -> p c l", l=L)[:, :, lo:hi],
                xdram[:].rearrange("(c p) l -> p c l", p=128)[:, :, lo:hi])

        # startup-critical order
        big_dma(xk_sb, xkT, 0, 512)
        for c in range(NEC):
            nc.sync.dma_start(
                xq_sb[c][:, 0:1024], xqT[c * 128:(c + 1) * 128, 0:1024])
        big_dma(xk_sb, xkT, 512, 1024)
        big_dma(xv_sb, xvT, 0, 512)
        big_dma(xv_sb, xvT, 512, 1024)
        nc.sync.dma_start(wt_v[:, :, 1, :], wT_v[:, :, 1, :])
        big_dma(xk_sb, xkT, 1024, 2048)
        big_dma(xv_sb, xvT, 1024, 2048)

        wo = const.tile([128, 2 * EMBED], f16)
        nc.sync.dma_start(
            wo[:].rearrange("p (g e) -> p g e", g=2),
            woT[:].rearrange("(g p) e -> p g e", p=128),
        )

        for c in range(NEC):
            nc.sync.dma_start(
                xq_sb[c][:, 1024:2048], xqT[c * 128:(c + 1) * 128, 1024:2048])

        # ---- persistent activations ---------------------------------------
        # qt[qcp]: [e-group g partitions (2 heads x 64d), free g*1024 + q]
        qt = [big.tile([128, 2048], f16, name=f"qt{i}") for i in range(2)]
        # ktt[qc]: [d partitions, g*512 + k-local]
        ktt = [big.tile([128, 1024], f16, name=f"ktt{i}") for i in range(4)]
        # vaug[kt]: [k partitions, 4h x (64d + ones)]
        vaug = [big.tile([128, 4 * 65], f16, name=f"vaug{i}") for i in range(NKT)]
        # ctxT[qcp][g]: [c partitions (2 heads x 64d), q 1024]
        ctxT = [[big.tile([128, 1024], f16, name=f"ctxT{i}{g}") for g in range(2)]
                for i in range(2)]

        # ones columns of vaug (denominator trick), one strided memset per kt
        for kt in range(NKT):
            nc.gpsimd.memset(
                vaug[kt][:].rearrange("p (h x) -> p x h", x=65)[:, 64:65, :], 1.0)

        # ---- PE warmup during the DMA-bound prologue -----------------------
        warm = pp_m.tile([128, 512], f32, tag="m")
        for i in range(24):
            nc.tensor.matmul(
                warm[:, 0:128], lhsT=idn[:], rhs=idn[:],
                start=(i == 0), stop=(i == 23))

        # ---- projection chains (as micro-piece closures) --------------------
        def chain_pieces(xap, qc, g, write):
            """Returns piece closures: [3 mms], [3 mms], [2 mms + evict]."""
            state = {}

            def mms(c0, c1):
                def _p():
                    if c0 == 0:
                        state["ps"] = pp_m.tile([128, 512], f32, tag="m",
                                                name="ps")
                    ps = state["ps"]
                    for c in range(c0, c1):
                        nc.tensor.matmul(
                            ps[:],
                            lhsT=wt[:, c * ES + g * 128: c * ES + (g + 1) * 128],
                            rhs=xap(c, qc),
                            start=(c == 0), stop=(c == NEC - 1))
                    if c1 == NEC:
                        write(state["ps"])
                return _p

            return [mms(0, 3), mms(3, 6), mms(6, 8)]

        def xq_ap(c, qc):
            return xq_sb[c][:, qc * 512:(qc + 1) * 512]

        def chain_q(qc, g):
            def write(ps):
                nc.vector.tensor_scalar_add(
                    qt[qc // 2][:, g * 1024 + (qc % 2) * 512:
                                g * 1024 + (qc % 2 + 1) * 512],
                    ps[:], bq[:, g:g + 1])
            return chain_pieces(xq_ap, qc, g, write)

        def chain_k(qc, g):
            def write(ps):
                nc.vector.tensor_scalar_add(
                    ktt[qc][:, g * 512:(g + 1) * 512], ps[:], bq[:, g:g + 1])
            return chain_pieces(xk_ap, qc, g, write)

        def chain_v(qc, g):
            """k/q pieces plus V transpose pieces into vaug."""
            state = {}

            def write(ps):
                vtt = work.tile([128, 512], f16, tag="vtt", bufs=2)
                nc.vector.tensor_scalar_add(vtt[:], ps[:], bq[:, g:g + 1])
                state["vtt"] = vtt

            pieces = chain_pieces(xv_ap, qc, g, write)

            def tp_piece(j0):
                def _p():
                    vtt = state["vtt"]
                    for j in (j0, j0 + 1):
                        tp = pp_m.tile([128, 128], f16, tag="m", name="tp")
                        nc.tensor.transpose(
                            tp[:], vtt[:, j * 128:(j + 1) * 128], idn[:])
                        nc.vector.tensor_copy(
                            vaug[qc * 4 + j][:, (2 * g) * 65:(2 * g + 2) * 65]
                            .rearrange("p (h x) -> p h x", h=2)[:, :, 0:64],
                            tp[:].rearrange("p (h x) -> p h x", h=2))
                return _p

            return pieces + [tp_piece(0), tp_piece(2)]

        # ---- attention pass: one e-group (2 heads) over one qcp ------------
        inv_sqrt_e = 1.0 / 32.0

        def make_pass(qcp, g, last=False):
            """Returns (S, P): scores/exp chunks and pv/finalize chunks."""
            state = {}

            def scores_chunk(kt, hh):
                sps = pp_s.tile([128, 1024], f32, tag="s")
                off = hh * 64
                for half in range(2):
                    nc.tensor.matmul(
                        sps[:, half * 512:(half + 1) * 512],
                        lhsT=ktt[kt // 4][
                            off:off + 64,
                            g * 512 + (kt % 4) * 128: g * 512 + (kt % 4 + 1) * 128],
                        rhs=qt[qcp][off:off + 64,
                                    g * 1024 + half * 512: g * 1024 + (half + 1) * 512],
                        start=True, stop=True)
                pt = work.tile([128, 1024], f16, tag="pt", bufs=6)
                nc.scalar.activation(
                    pt[:], sps[:], mybir.ActivationFunctionType.Exp,
                    scale=inv_sqrt_e)
                state[("pt", kt, hh)] = pt

            def pv_chunk(kt, hh):
                # one PSUM start/stop per bank per pass (zero-region rule):
                # bank b of ctx starts at (kt0, hh0, q8=3b), stops at
                # (kt15, hh1, q8 = 2/5/7).
                if kt == 0 and hh == 0:
                    state["ctx"] = pp_ctx.tile(
                        [128, 1536], f32, tag="ctx", name="ctxp")
                ctxp = state["ctx"]
                pt = state.pop(("pt", kt, hh))
                h = 2 * g + hh
                for q8 in range(8):
                    o = ctx_off(q8, hh)
                    nc.tensor.matmul(
                        ctxp[:, o: o + 65],
                        lhsT=pt[:, q8 * 128:(q8 + 1) * 128],
                        rhs=vaug[kt][:, h * 65:(h + 1) * 65],
                        start=(kt == 0 and hh == 0 and q8 % 3 == 0),
                        stop=(kt == NKT - 1 and hh == 1 and q8 in (2, 5, 7)))

            def fin_batch():
                """Batched finalize: 3 per-bank recips + 3 per-bank muls
                (rec broadcast via 0-stride), then 8 async DMA transposes
                into ctxT. Short critical chain at the pass boundary."""
                ctxp = state["ctx"]
                rec = work.tile([128, 16], f32, tag="rec", bufs=2, name="rec")
                cn = work.tile([128, 1024], f16, tag="cn", bufs=2, name="cn")
                for b in range(3):
                    npair = 3 if b < 2 else 2
                    den_view = ctxp[:, b * 512: b * 512 + npair * 130].rearrange(
                        "p (r h x) -> p r h x", h=2, x=65)[:, :, :, 64:65]
                    nc.vector.reciprocal(
                        rec[:, b * 6: b * 6 + npair * 2].rearrange(
                            "p (r h x) -> p r h x", h=2, x=1),
                        den_view)
                for b in range(3):
                    npair = 3 if b < 2 else 2
                    nc.vector.tensor_mul(
                        cn[:, b * 3 * 128: (b * 3 + npair) * 128].rearrange(
                            "p (r h x) -> p r h x", h=2, x=64),
                        ctxp[:, b * 512: b * 512 + npair * 130].rearrange(
                            "p (r h x) -> p r h x", h=2, x=65)[:, :, :, 0:64],
                        rec[:, b * 6: b * 6 + npair * 2].rearrange(
                            "p (r h x) -> p r h x", h=2, x=1).broadcast_to(
                            [128, npair, 2, 64]))
                if last:
                    _CACHE["last_cn"] = cn
                    return
                for q8 in range(8):
                    nc.sync.dma_start_transpose(
                        ctxT[qcp][g][:, q8 * 128:(q8 + 1) * 128],
                        cn[:, q8 * 128:(q8 + 1) * 128])

            S = [(lambda kt=kt, hh=hh: scores_chunk(kt, hh))
                 for kt in range(NKT) for hh in range(2)]
            P = [("pv", (lambda kt=kt, hh=hh: pv_chunk(kt, hh)))
                 for kt in range(NKT) for hh in range(2)]
            P += [("fin", fin_batch)]
            return S, P

        # ---- output projection ----------------------------------------------
        def make_outproj(qcp, pool_tags, evict_split=False):
            """Yields single-matmul pieces: (g0 mm) then (g1 mm + evict)."""
            state = {}

            def part(lt, oc, g, slot_i):
                if g == 0:
                    pool, tag = pool_tags[slot_i % len(pool_tags)]
                    state["ops"] = pool.tile([128, 512], f32, tag=tag,
                                             name="ops")
                ops = state["ops"]
                nc.tensor.matmul(
                    ops[:],
                    lhsT=ctxT[qcp][g][:, lt * 128:(lt + 1) * 128],
                    rhs=wo[:, g * EMBED + oc * 512: g * EMBED + (oc + 1) * 512],
                    start=(g == 0), stop=(g == 1))
                if g == 1:
                    if oc == 0:
                        state["ot"] = work.tile([128, 1024], f16, tag="ot",
                                                bufs=4, name="ot")
                    ot = state["ot"]
                    if evict_split and slot_i % 2 == 1:
                        nc.scalar.copy(ot[:, oc * 512:(oc + 1) * 512], ops[:])
                    else:
                        nc.vector.tensor_copy(
                            ot[:, oc * 512:(oc + 1) * 512], ops[:])
                    if oc == 1:
                        lt_g = qcp * 8 + lt
                        nc.sync.dma_start(
                            out[lt_g * 128:(lt_g + 1) * 128, :], ot[:])

            i = 0
            for lt in range(8):
                for oc in range(2):
                    for g in range(2):
                        yield (lambda lt=lt, oc=oc, g=g, i=i: part(lt, oc, g, i))
                    i += 1

        # qcp1 g-split: g0 partials evicted to ot0 staging during pass 3,
        # g1 matmuls + adds + DMA at the tail.
        ot0 = [work.tile([128, 1024], f16, tag="ot0", bufs=8, name=f"ot0_{lt}")
               for lt in range(8)]

        def make_op_g0(qcp, pool_tags):
            def piece(lt, oc, slot_i):
                pool, tag = pool_tags[slot_i % len(pool_tags)]
                ops = pool.tile([128, 512], f32, tag=tag, name="ops0")
                nc.tensor.matmul(
                    ops[:],
                    lhsT=ctxT[qcp][0][:, lt * 128:(lt + 1) * 128],
                    rhs=wo[:, oc * 512:(oc + 1) * 512],
                    start=True, stop=True)
                nc.vector.tensor_copy(ot0[lt][:, oc * 512:(oc + 1) * 512],
                                      ops[:])
                if oc == 1:
                    nc.sync.dma_start(out2[lt * 128:(lt + 1) * 128, :],
                                      ot0[lt][:])

            i = 0
            for lt in range(8):
                for oc in range(2):
                    yield (lambda lt=lt, oc=oc, i=i: piece(lt, oc, i))
                    i += 1

        def make_op_g1(qcp, pool_tags):
            state = {}

            def piece(lt, oc, slot_i):
                pool, tag = pool_tags[slot_i % len(pool_tags)]
                ops = pool.tile([128, 512], f32, tag=tag, name="ops1")
                nc.tensor.matmul(
                    ops[:],
                    lhsT=ctxT[qcp][1][:, lt * 128:(lt + 1) * 128],
                    rhs=wo[:, EMBED + oc * 512: EMBED + (oc + 1) * 512],
                    start=True, stop=True)
                if oc == 0:
                    state["ot"] = work.tile([128, 1024], f16, tag="ot",
                                            bufs=4, name="ot")
                ot = state["ot"]
                if slot_i % 2 == 1:
                    nc.scalar.copy(ot[:, oc * 512:(oc + 1) * 512], ops[:])
                else:
                    nc.vector.tensor_copy(ot[:, oc * 512:(oc + 1) * 512],
                                          ops[:])
                if oc == 1:
                    lt_g = qcp * 8 + lt
                    nc.sync.dma_start(
                        out[lt_g * 128:(lt_g + 1) * 128, :], ot[:])

            i = 0
            for lt in range(8):
                for oc in range(2):
                    yield (lambda lt=lt, oc=oc, i=i: piece(lt, oc, i))
                    i += 1

        # ---- emission schedule ----------------------------------------------
        # Pass order (0,g0),(1,g0),(0,g1),(1,g1): passes 1-2 share the g0
        # K/V tensors so all g1 projection chains defer to later passes,
        # balancing PE load against the ACT-bound exp stream in every pass.
        for piece in chain_k(0, 0):
            piece()
        # q00/q10 chunk-paced through the (still unused) scores psum slots so
        # their matmuls track the xq chunk DMAs
        ps_q = [pp_s.tile([128, 512], f32, tag="s", name=f"psq{i}")
                for i in range(2)]
        for c in range(NEC):
            for i, qc in enumerate((0, 1)):
                nc.tensor.matmul(
                    ps_q[i][:],
                    lhsT=wt[:, c * ES: c * ES + 128],
                    rhs=xq_sb[c][:, qc * 512:(qc + 1) * 512],
                    start=(c == 0), stop=(c == NEC - 1))
        for i, qc in enumerate((0, 1)):
            nc.vector.tensor_scalar_add(
                qt[0][:, qc * 512:(qc + 1) * 512], ps_q[i][:], bq[:, 0:1])


        S_all, P_all = [], []
        for (qcp, g) in [(0, 0), (1, 0), (0, 1), (1, 1)]:
            S, P = make_pass(qcp, g, last=(qcp == 1 and g == 1))
            S_all += S
            P_all += P

        def log(label):
            nm = nc.get_next_instruction_name()  # peeks+consumes one id
            EMITLOG.append((label, int(nm.split("-")[1])))

        S_all[0]()
        log("S0")
        S_all[1]()
        log("S1")
        for piece in chain_v(0, 0):
            piece()
        log("v00")

        from collections import deque
        pieces = deque()
        for cl in (chain_k(1, 0), chain_v(1, 0),
                   chain_k(2, 0), chain_k(3, 0), chain_v(2, 0), chain_v(3, 0),
                   chain_q(2, 0), chain_q(3, 0),
                   chain_k(0, 1), chain_k(1, 1), chain_k(2, 1), chain_k(3, 1),
                   chain_v(0, 1), chain_v(1, 1),
                   chain_q(0, 1), chain_q(1, 1),
                   chain_v(2, 1), chain_v(3, 1),
                   chain_q(2, 1), chain_q(3, 1)):
            pieces.extend(cl)

        def drain(n):
            for _ in range(n):
                if pieces:
                    pieces.popleft()()

        # P-step indexing: pass p occupies [33p, 33p+32]; 32 pv + 1 fin batch.
        # op-C (qcp1 g0 partials) woven into pass 3; op0 (qcp0, full octiles)
        # into pass 4; op-D (qcp1 g1 + adds) at the tail.
        opC = list(make_op_g0(1, [(pp_m, "m")]))
        opC_at = {67 + 2 * j: cl for j, cl in enumerate(opC)}
        op0 = list(make_outproj(0, [(pp_m, "m")]))
        op0_at = {100 + j: cl for j, cl in enumerate(op0)}

        si = 2
        pv_done = 0
        for pi, (kind, p) in enumerate(P_all):
            p()
            log(f"P{pi}:{kind}")
            if kind == "pv":
                pv_done += 1
            if pi in opC_at:
                opC_at[pi]()
                log(f"opC@{pi}")
            if pi in op0_at:
                op0_at[pi]()
                log(f"op0@{pi}")
            if kind == "pv":
                drain(2 if pi < 4 else 1)
                log(f"drain@{pi}")
            while si < len(S_all) and si < pv_done + 5:
                S_all[si]()
                log(f"S{si}")
                si += 1
        while si < len(S_all):
            S_all[si]()
            si += 1
        drain(len(pieces))
        log("tail-start")

        # tail: per-lt, PE-transpose the last pass's cn slice into ctxT,
        # immediately followed by that lt's op-D pieces.
        cn_last = _CACHE.pop("last_cn")
        opD = list(make_op_g1(1, [(pp_s, "s"), (pp_s, "s")]))
        for lt in range(8):
            tp = pp_m.tile([128, 128], f16, tag="m", name="tpd")
            nc.tensor.transpose(
                tp[:], cn_last[:, lt * 128:(lt + 1) * 128], idn[:])
            nc.vector.tensor_copy(ctxT[1][1][:, lt * 128:(lt + 1) * 128], tp[:])
            opD[2 * lt]()
            opD[2 * lt + 1]()

    nc.compile()
    return nc


def kernel(query, key, values, W1, b1):
    from concourse.bass_utils import run_bass_kernel_spmd

    if "nc" not in _CACHE:
        _CACHE["nc"] = _gen_kernel()
    nc = _CACHE["nc"]

    query = np.asarray(query, dtype=np.float32)
    key = np.asarray(key, dtype=np.float32)
    values = np.asarray(values, dtype=np.float32)
    W1 = np.asarray(W1, dtype=np.float32)
    b1 = np.asarray(b1, dtype=np.float32)

    xT = {}
    for b in range(B):
        xT[("q", b)] = np.ascontiguousarray(query[b].T).astype(np.float16)
        xT[("k", b)] = np.ascontiguousarray(key[b].T).astype(np.float16)
        xT[("v", b)] = np.ascontiguousarray(values[b].T).astype(np.float16)

    in_maps = []
    for core in range(N_CORES):
        b = core // HPC
        hg = core % HPC
        sl = slice(hg * ES, (hg + 1) * ES)
        in_maps.append({
            "xqT": xT[("q", b)],
            "xkT": xT[("k", b)],
            "xvT": xT[("v", b)],
            "wT": np.ascontiguousarray(W1[sl, :].T).astype(np.float16),
            "woT": np.ascontiguousarray(W1[:, sl].T).astype(np.float16),
            "bqkv": np.ascontiguousarray(b1[sl].reshape(2, 128).T),
        })

    res = run_bass_kernel_spmd(
        nc, in_maps, core_ids=list(range(N_CORES)),
        trace=bool(_CACHE.get("trace", False)))
    _CACHE["last_results"] = res

    output = np.empty((B, L, EMBED), dtype=np.float32)
    for b in range(B):
        acc = res.results[b * HPC]["out"].astype(np.float32)
        acc[L // 2:] += res.results[b * HPC]["out2"].astype(np.float32)
        for hg in range(1, HPC):
            acc += res.results[b * HPC + hg]["out"].astype(np.float32)
            acc[L // 2:] += res.results[b * HPC + hg]["out2"].astype(np.float32)
        output[b] = acc + b1[None, :]
    return output


# revision 37
# speedup vs baseline: 1.0052x; 1.0052x over previous
"""Multi-headed attention (B=2, L=2048, E=1024, H=16) on 8 trn2 cores.

Sharding: batch (2) x head-groups (4) -> 8 cores. Each core computes 4 heads
of one batch element end-to-end (QKV projection, attention, partial output
projection); host sums the per-head-group partial outputs (out + out2) per
batch and adds the final bias.

All data is fp16 (host converts); PSUM accumulates fp32. Design, driven by
the TimelineSim cost model (matmul cost = out free size; exp is ACT-only):
  - QKV projections: W-slice stationary, x^T moving (N=512 chains).
  - Scores S^T [k, q] per head into [128,1024] PSUM, one wide exp per
    (head, k-tile) on ACT straight to fp16 SBUF (1038ns each; ACT busy
    ~134us is the co-critical path with PE ~140us).
  - PV flipped to out[q, d]: pt (exp'd scores) stationary, V [k, 64+1]
    moving (N=65; ones column accumulates the softmax denominator) -- half
    the PE rows of the N=q orientation.
  - Normalization fused into the ctx eviction (reciprocal + 0-stride
    broadcast multiply, batched per PSUM bank); ctx^T via DMA transpose
    (mid-kernel) or PE transpose (final pass, avoids DMA latency).
  - Attention runs in 4 passes of 2 heads ordered (qcp0,g0),(qcp1,g0),
    (qcp0,g1),(qcp1,g1) so passes 1-2 reuse g0 K/V and all g1 projection
    chains spread into later passes, balancing PE load per pass.
  - PSUM (8 banks): scores 2x[128,1024] (4) + ctx regions 130-wide pairs
    (3) + misc (1); one start/stop per bank per pass (zero-region rule).
  - Emission is software-pipelined: scores/exp run ~2 chunks ahead of PV;
    projection chains weave in as ~3-matmul micro-pieces; out-projection:
    qcp0 whole octiles in pass 4, qcp1 g0-part DMA'd to out2 during pass 3
    (host adds), g1-part at the tail woven with per-lt PE transposes.
"""

import numpy as np

EMBED = 1024
HEADS = 16
HD = 64
B = 2
L = 2048
N_CORES = 8
HPC = 4              # heads per core
ES = HPC * HD        # 256: e-slice width per core
NEC = EMBED // 128   # 8 embed chunks
NKT = L // 128       # 16 k-tiles

_CACHE = {}
EMITLOG = []


def _gen_kernel():
    from contextlib import ExitStack

    import concourse.mybir as mybir
    import concourse.tile as tile
    from concourse import bacc
    from concourse.masks import make_identity

    dt = mybir.dt
    f32 = dt.float32
    f16 = dt.float16

    nc = bacc.Bacc("TRN2", target_bir_lowering=False)

    xqT = nc.dram_tensor("xqT", [EMBED, L], f16, kind="ExternalInput")
    xkT = nc.dram_tensor("xkT", [EMBED, L], f16, kind="ExternalInput")
    xvT = nc.dram_tensor("xvT", [EMBED, L], f16, kind="ExternalInput")
    wT = nc.dram_tensor("wT", [EMBED, ES], f16, kind="ExternalInput")
    woT = nc.dram_tensor("woT", [ES, EMBED], f16, kind="ExternalInput")
    bqkv = nc.dram_tensor("bqkv", [128, 2], f32, kind="ExternalInput")
    out = nc.dram_tensor("out", [L, EMBED], f16, kind="ExternalOutput")
    out2 = nc.dram_tensor("out2", [L // 2, EMBED], f16, kind="ExternalOutput")

    # ctx region offset: (q8, hh) -> free offset in the 3-bank ctx tile.
    # Pairs of 130 (two heads x 65), 3 pairs per 512-f32 bank, no straddle.
    def ctx_off(q8, hh):
        return (q8 // 3) * 512 + (q8 % 3) * 130 + hh * 65

    with tile.TileContext(nc) as tc, ExitStack() as ctx:
        const = ctx.enter_context(tc.tile_pool(name="const", bufs=1))
        xin = ctx.enter_context(tc.tile_pool(name="xin", bufs=1))
        big = ctx.enter_context(tc.tile_pool(name="big", bufs=1))
        work = ctx.enter_context(tc.tile_pool(name="work", bufs=2))
        # PSUM budget (8 banks): scores 2x2 + ctx 3 + misc 1
        pp_s = ctx.enter_context(tc.tile_pool(name="pp_s", bufs=2, space="PSUM"))
        pp_ctx = ctx.enter_context(tc.tile_pool(name="pp_ctx", bufs=1, space="PSUM"))
        pp_m = ctx.enter_context(tc.tile_pool(name="pp_m", bufs=1, space="PSUM"))

        # ---- constants (g0 half of wt first; wo deferred) ------------------
        wt = const.tile([128, NEC * ES], f16)
        wt_v = wt[:].rearrange("p (c g e) -> p c g e", c=NEC, g=2)
        wT_v = wT[:].rearrange("(c p) (g e) -> p c g e", p=128, g=2)
        nc.sync.dma_start(wt_v[:, :, 0, :], wT_v[:, :, 0, :])
        bq = const.tile([128, 2], f32)
        nc.sync.dma_start(bq[:], bqkv[:])

        idn = const.tile([128, 128], f16)
        make_identity(nc, idn[:])

        # ---- x staging (fp16; xq chunk-tiles, xk/xv single tiles) ----------
        xq_sb = xin.tile([128, NEC * L], f16, name="xq")
        xk_sb = xin.tile([128, NEC * L], f16, name="xk")
        xv_sb = xin.tile([128, NEC * L], f16, name="xv")

        def xq_ap(c, qc):
            return xq_sb[:, c * L + qc * 512:(c * L) + (qc + 1) * 512]

        def xk_ap(c, qc):
            return xk_sb[:, c * L + qc * 512:(c * L) + (qc + 1) * 512]

        def xv_ap(c, qc):
            return xv_sb[:, c * L + qc * 512:(c * L) + (qc + 1) * 512]

        def big_dma(xsb, xdram, lo, hi):
            nc.sync.dma_start(
                xsb[:].rearrange("p (c l) -> p c l", l=L)[:, :, lo:hi],
                xdram[:].rearrange("(c p) l -> p c l", p=128)[:, :, lo:hi])

        # startup-critical order
        big_dma(xk_sb, xkT, 0, 512)
        big_dma(xq_sb, xqT, 0, 512)
        big_dma(xq_sb, xqT, 512, 1024)
        big_dma(xv_sb, xvT, 0, 512)
        big_dma(xk_sb, xkT, 512, 1024)
        big_dma(xv_sb, xvT, 512, 1024)
        nc.sync.dma_start(wt_v[:, :, 1, :], wT_v[:, :, 1, :])
        big_dma(xk_sb, xkT, 1024, 2048)
        big_dma(xv_sb, xvT, 1024, 2048)

        wo = const.tile([128, 2 * EMBED], f16)
        nc.sync.dma_start(
            wo[:].rearrange("p (g e) -> p g e", g=2),
            woT[:].rearrange("(g p) e -> p g e", p=128),
        )

        big_dma(xq_sb, xqT, 1024, 2048)

        # ---- persistent activations ---------------------------------------
        # qt[qcp]: [e-group g partitions (2 heads x 64d), free g*1024 + q]
        qt = [big.tile([128, 2048], f16, name=f"qt{i}") for i in range(2)]
        # ktt[qc]: [d partitions, g*512 + k-local]
        ktt = [big.tile([128, 1024], f16, name=f"ktt{i}") for i in range(4)]
        # vaug[kt]: [k partitions, 4h x (64d + ones)]
        vaug = [big.tile([128, 4 * 65], f16, name=f"vaug{i}") for i in range(NKT)]
        # ctxT[qcp][g]: [c partitions (2 heads x 64d), q 1024]
        ctxT = [[big.tile([128, 1024], f16, name=f"ctxT{i}{g}") for g in range(2)]
                for i in range(2)]

        # ones columns of vaug (denominator trick), one strided memset per kt
        for kt in range(NKT):
            nc.gpsimd.memset(
                vaug[kt][:].rearrange("p (h x) -> p x h", x=65)[:, 64:65, :], 1.0)

        # ---- PE warmup during the DMA-bound prologue -----------------------
        warm = pp_m.tile([128, 512], f32, tag="m")
        for i in range(24):
            nc.tensor.matmul(
                warm[:, 0:128], lhsT=idn[:], rhs=idn[:],
                start=(i == 0), stop=(i == 23))

        # ---- projection chains (as micro-piece closures) --------------------
        def chain_pieces(xap, qc, g, write):
            """Returns piece closures: [3 mms], [3 mms], [2 mms + evict]."""
            state = {}

            def mms(c0, c1):
                def _p():
                    if c0 == 0:
                        state["ps"] = pp_m.tile([128, 512], f32, tag="m",
                                                name="ps")
                    ps = state["ps"]
                    for c in range(c0, c1):
                        nc.tensor.matmul(
                            ps[:],
                            lhsT=wt[:, c * ES + g * 128: c * ES + (g + 1) * 128],
                            rhs=xap(c, qc),
                            start=(c == 0), stop=(c == NEC - 1))
                    if c1 == NEC:
                        write(state["ps"])
                return _p

            return [mms(0, 3), mms(3, 6), mms(6, 8)]

        def chain_q(qc, g):
            def write(ps):
                nc.vector.tensor_scalar_add(
                    qt[qc // 2][:, g * 1024 + (qc % 2) * 512:
                                g * 1024 + (qc % 2 + 1) * 512],
                    ps[:], bq[:, g:g + 1])
            return chain_pieces(xq_ap, qc, g, write)

        def chain_k(qc, g):
            def write(ps):
                nc.vector.tensor_scalar_add(
                    ktt[qc][:, g * 512:(g + 1) * 512], ps[:], bq[:, g:g + 1])
            return chain_pieces(xk_ap, qc, g, write)

        def chain_v(qc, g):
            """k/q pieces plus V transpose pieces into vaug."""
            state = {}

            def write(ps):
                vtt = work.tile([128, 512], f16, tag="vtt", bufs=2)
                nc.vector.tensor_scalar_add(vtt[:], ps[:], bq[:, g:g + 1])
                state["vtt"] = vtt

            pieces = chain_pieces(xv_ap, qc, g, write)

            def tp_piece(j0):
                def _p():
                    vtt = state["vtt"]
                    for j in (j0, j0 + 1):
                        tp = pp_m.tile([128, 128], f16, tag="m", name="tp")
                        nc.tensor.transpose(
                            tp[:], vtt[:, j * 128:(j + 1) * 128], idn[:])
                        nc.vector.tensor_copy(
                            vaug[qc * 4 + j][:, (2 * g) * 65:(2 * g + 2) * 65]
                            .rearrange("p (h x) -> p h x", h=2)[:, :, 0:64],
                            tp[:].rearrange("p (h x) -> p h x", h=2))
                return _p

            return pieces + [tp_piece(0), tp_piece(2)]

        # ---- attention pass: one e-group (2 heads) over one qcp ------------
        inv_sqrt_e = 1.0 / 32.0

        def make_pass(qcp, g, last=False):
            """Returns (S, P): scores/exp chunks and pv/finalize chunks."""
            state = {}

            def scores_chunk(kt, hh):
                sps = pp_s.tile([128, 1024], f32, tag="s")
                off = hh * 64
                for half in range(2):
                    nc.tensor.matmul(
                        sps[:, half * 512:(half + 1) * 512],
                        lhsT=ktt[kt // 4][
                            off:off + 64,
                            g * 512 + (kt % 4) * 128: g * 512 + (kt % 4 + 1) * 128],
                        rhs=qt[qcp][off:off + 64,
                                    g * 1024 + half * 512: g * 1024 + (half + 1) * 512],
                        start=True, stop=True)
                pt = work.tile([128, 1024], f16, tag="pt", bufs=6)
                nc.scalar.activation(
                    pt[:], sps[:], mybir.ActivationFunctionType.Exp,
                    scale=inv_sqrt_e)
                state[("pt", kt, hh)] = pt

            def pv_chunk(kt, hh):
                # one PSUM start/stop per bank per pass (zero-region rule):
                # bank b of ctx starts at (kt0, hh0, q8=3b), stops at
                # (kt15, hh1, q8 = 2/5/7).
                if kt == 0 and hh == 0:
                    state["ctx"] = pp_ctx.tile(
                        [128, 1536], f32, tag="ctx", name="ctxp")
                ctxp = state["ctx"]
                pt = state.pop(("pt", kt, hh))
                h = 2 * g + hh
                for q8 in range(8):
                    o = ctx_off(q8, hh)
                    nc.tensor.matmul(
                        ctxp[:, o: o + 65],
                        lhsT=pt[:, q8 * 128:(q8 + 1) * 128],
                        rhs=vaug[kt][:, h * 65:(h + 1) * 65],
                        start=(kt == 0 and hh == 0 and q8 % 3 == 0),
                        stop=(kt == NKT - 1 and hh == 1 and q8 in (2, 5, 7)))

            def fin_batch():
                """Batched finalize: 3 per-bank recips + 3 per-bank muls
                (rec broadcast via 0-stride), then 8 async DMA transposes
                into ctxT. Short critical chain at the pass boundary."""
                ctxp = state["ctx"]
                rec = work.tile([128, 16], f32, tag="rec", bufs=2, name="rec")
                cn = work.tile([128, 1024], f16, tag="cn", bufs=2, name="cn")
                for b in range(3):
                    npair = 3 if b < 2 else 2
                    den_view = ctxp[:, b * 512: b * 512 + npair * 130].rearrange(
                        "p (r h x) -> p r h x", h=2, x=65)[:, :, :, 64:65]
                    nc.vector.reciprocal(
                        rec[:, b * 6: b * 6 + npair * 2].rearrange(
                            "p (r h x) -> p r h x", h=2, x=1),
                        den_view)
                for b in range(3):
                    npair = 3 if b < 2 else 2
                    nc.vector.tensor_mul(
                        cn[:, b * 3 * 128: (b * 3 + npair) * 128].rearrange(
                            "p (r h x) -> p r h x", h=2, x=64),
                        ctxp[:, b * 512: b * 512 + npair * 130].rearrange(
                            "p (r h x) -> p r h x", h=2, x=65)[:, :, :, 0:64],
                        rec[:, b * 6: b * 6 + npair * 2].rearrange(
                            "p (r h x) -> p r h x", h=2, x=1).broadcast_to(
                            [128, npair, 2, 64]))
                if last:
                    _CACHE["last_cn"] = cn
                    return
                for q8 in range(8):
                    nc.sync.dma_start_transpose(
                        ctxT[qcp][g][:, q8 * 128:(q8 + 1) * 128],
                        cn[:, q8 * 128:(q8 + 1) * 128])

            S = [(lambda kt=kt, hh=hh: scores_chunk(kt, hh))
                 for kt in range(NKT) for hh in range(2)]
            P = [("pv", (lambda kt=kt, hh=hh: pv_chunk(kt, hh)))
                 for kt in range(NKT) for hh in range(2)]
            P += [("fin", fin_batch)]
            return S, P

        # ---- output projection ----------------------------------------------
        def make_outproj(qcp, pool_tags, evict_split=False):
            """Yields single-matmul pieces: (g0 mm) then (g1 mm + evict)."""
            state = {}

            def part(lt, oc, g, slot_i):
                if g == 0:
                    pool, tag = pool_tags[slot_i % len(pool_tags)]
                    state["ops"] = pool.tile([128, 512], f32, tag=tag,
                                             name="ops")
                ops = state["ops"]
                nc.tensor.matmul(
                    ops[:],
                    lhsT=ctxT[qcp][g][:, lt * 128:(lt + 1) * 128],
                    rhs=wo[:, g * EMBED + oc * 512: g * EMBED + (oc + 1) * 512],
                    start=(g == 0), stop=(g == 1))
                if g == 1:
                    if oc == 0:
                        state["ot"] = work.tile([128, 1024], f16, tag="ot",
                                                bufs=4, name="ot")
                    ot = state["ot"]
                    if evict_split and slot_i % 2 == 1:
                        nc.scalar.copy(ot[:, oc * 512:(oc + 1) * 512], ops[:])
                    else:
                        nc.vector.tensor_copy(
                            ot[:, oc * 512:(oc + 1) * 512], ops[:])
                    if oc == 1:
                        lt_g = qcp * 8 + lt
                        nc.sync.dma_start(
                            out[lt_g * 128:(lt_g + 1) * 128, :], ot[:])

            i = 0
            for lt in range(8):
                for oc in range(2):
                    for g in range(2):
                        yield (lambda lt=lt, oc=oc, g=g, i=i: part(lt, oc, g, i))
                    i += 1

        # qcp1 g-split: g0 partials evicted to ot0 staging during pass 3,
        # g1 matmuls + adds + DMA at the tail.
        ot0 = [work.tile([128, 1024], f16, tag="ot0", bufs=8, name=f"ot0_{lt}")
               for lt in range(8)]

        def make_op_g0(qcp, pool_tags):
            def piece(lt, oc, slot_i):
                pool, tag = pool_tags[slot_i % len(pool_tags)]
                ops = pool.tile([128, 512], f32, tag=tag, name="ops0")
                nc.tensor.matmul(
                    ops[:],
                    lhsT=ctxT[qcp][0][:, lt * 128:(lt + 1) * 128],
                    rhs=wo[:, oc * 512:(oc + 1) * 512],
                    start=True, stop=True)
                nc.vector.tensor_copy(ot0[lt][:, oc * 512:(oc + 1) * 512],
                                      ops[:])
                if oc == 1:
                    nc.sync.dma_start(out2[lt * 128:(lt + 1) * 128, :],
                                      ot0[lt][:])

            i = 0
            for lt in range(8):
                for oc in range(2):
                    yield (lambda lt=lt, oc=oc, i=i: piece(lt, oc, i))
                    i += 1

        def make_op_g1(qcp, pool_tags):
            state = {}

            def piece(lt, oc, slot_i):
                pool, tag = pool_tags[slot_i % len(pool_tags)]
                ops = pool.tile([128, 512], f32, tag=tag, name="ops1")
                nc.tensor.matmul(
                    ops[:],
                    lhsT=ctxT[qcp][1][:, lt * 128:(lt + 1) * 128],
                    rhs=wo[:, EMBED + oc * 512: EMBED + (oc + 1) * 512],
                    start=True, stop=True)
                if oc == 0:
                    state["ot"] = work.tile([128, 1024], f16, tag="ot",
                                            bufs=4, name="ot")
                ot = state["ot"]
                if slot_i % 2 == 1:
                    nc.scalar.copy(ot[:, oc * 512:(oc + 1) * 512], ops[:])
                else:
                    nc.vector.tensor_copy(ot[:, oc * 512:(oc + 1) * 512],
                                          ops[:])
                if oc == 1:
                    lt_g = qcp * 8 + lt
                    nc.sync.dma_start(
                        out[lt_g * 128:(lt_g + 1) * 128, :], ot[:])

            i = 0
            for lt in range(8):
                for oc in range(2):
                    yield (lambda lt=lt, oc=oc, i=i: piece(lt, oc, i))
                    i += 1

        # ---- emission schedule ----------------------------------------------
        # Pass order (0,g0),(1,g0),(0,g1),(1,g1): passes 1-2 share the g0
        # K/V tensors so all g1 projection chains defer to later passes,
        # balancing PE load against the ACT-bound exp stream in every pass.
        # interleave k00/q00 chain pieces so both track their DMAs; q00/q10
        # use the (still free) scores psum slots to run parallel to k00
        def chain_q_slot(qc):
            ps = pp_s.tile([128, 512], f32, tag="s", name="psq")
            for c in range(NEC):
                nc.tensor.matmul(
                    ps[:],
                    lhsT=wt[:, c * ES: c * ES + 128],
                    rhs=xq_ap(c, qc),
                    start=(c == 0), stop=(c == NEC - 1))
            nc.vector.tensor_scalar_add(
                qt[0][:, qc * 512:(qc + 1) * 512], ps[:], bq[:, 0:1])

        k00 = chain_k(0, 0)
        k00[0]()
        chain_q_slot(0)
        k00[1]()
        chain_q_slot(1)
        k00[2]()


        S_all, P_all = [], []
        for (qcp, g) in [(0, 0), (1, 0), (0, 1), (1, 1)]:
            S, P = make_pass(qcp, g, last=(qcp == 1 and g == 1))
            S_all += S
            P_all += P

        def log(label):
            nm = nc.get_next_instruction_name()  # peeks+consumes one id
            EMITLOG.append((label, int(nm.split("-")[1])))

        S_all[0]()
        log("S0")
        S_all[1]()
        log("S1")
        for piece in chain_v(0, 0):
            piece()
        log("v00")

        from collections import deque
        pieces = deque()
        for cl in (chain_k(1, 0), chain_v(1, 0),
                   chain_k(2, 0), chain_k(3, 0), chain_v(2, 0), chain_v(3, 0),
                   chain_q(2, 0), chain_q(3, 0),
                   chain_k(0, 1), chain_k(1, 1), chain_k(2, 1), chain_k(3, 1),
                   chain_v(0, 1), chain_v(1, 1),
                   chain_q(0, 1), chain_q(1, 1),
                   chain_v(2, 1), chain_v(3, 1),
                   chain_q(2, 1), chain_q(3, 1)):
            pieces.extend(cl)

        def drain(n):
            for _ in range(n):
                if pieces:
                    pieces.popleft()()

        # P-step indexing: pass p occupies [33p, 33p+32]; 32 pv + 1 fin batch.
        # op-C (qcp1 g0 partials) woven into pass 3; op0 (qcp0, full octiles)
        # into pass 4; op-D (qcp1 g1 + adds) at the tail.
        opC = list(make_op_g0(1, [(pp_m, "m")]))
        opC_at = {67 + 2 * j: cl for j, cl in enumerate(opC)}
        op0 = list(make_outproj(0, [(pp_m, "m")]))
        op0_at = {100 + j: cl for j, cl in enumerate(op0)}

        si = 2
        pv_done = 0
        for pi, (kind, p) in enumerate(P_all):
            p()
            log(f"P{pi}:{kind}")
            if kind == "pv":
                pv_done += 1
            if pi in opC_at:
                opC_at[pi]()
                log(f"opC@{pi}")
            if pi in op0_at:
                op0_at[pi]()
                log(f"op0@{pi}")
            if kind == "pv":
                drain(2 if pi < 4 else 1)
                log(f"drain@{pi}")
            while si < len(S_all) and si < pv_done + 5:
                S_all[si]()
                log(f"S{si}")
                si += 1
        while si < len(S_all):
            S_all[si]()
            si += 1
        drain(len(pieces))
        log("tail-start")

        # tail: per-lt, PE-transpose the last pass's cn slice into ctxT,
        # immediately followed by that lt's op-D pieces.
        cn_last = _CACHE.pop("last_cn")
        opD = list(make_op_g1(1, [(pp_s, "s"), (pp_s, "s"), (pp_ctx, "ctx")]))
        for lt in range(8):
            tp = pp_m.tile([128, 128], f16, tag="m", name="tpd")
            nc.tensor.transpose(
                tp[:], cn_last[:, lt * 128:(lt + 1) * 128], idn[:])
            nc.vector.tensor_copy(ctxT[1][1][:, lt * 128:(lt + 1) * 128], tp[:])
            opD[2 * lt]()
            opD[2 * lt + 1]()

    nc.compile()
    return nc


def kernel(query, key, values, W1, b1):
    from concourse.bass_utils import run_bass_kernel_spmd

    if "nc" not in _CACHE:
        _CACHE["nc"] = _gen_kernel()
    nc = _CACHE["nc"]

    query = np.asarray(query, dtype=np.float32)
    key = np.asarray(key, dtype=np.float32)
    values = np.asarray(values, dtype=np.float32)
    W1 = np.asarray(W1, dtype=np.float32)
    b1 = np.asarray(b1, dtype=np.float32)

    xT = {}
    for b in range(B):
        xT[("q", b)] = np.ascontiguousarray(query[b].T).astype(np.float16)
        xT[("k", b)] = np.ascontiguousarray(key[b].T).astype(np.float16)
        xT[("v", b)] = np.ascontiguousarray(values[b].T).astype(np.float16)

    in_maps = []
    for core in range(N_CORES):
        b = core // HPC
        hg = core % HPC
        sl = slice(hg * ES, (hg + 1) * ES)
        in_maps.append({
            "xqT": xT[("q", b)],
            "xkT": xT[("k", b)],
            "xvT": xT[("v", b)],
            "wT": np.ascontiguousarray(W1[sl, :].T).astype(np.float16),
            "woT": np.ascontiguousarray(W1[:, sl].T).astype(np.float16),
            "bqkv": np.ascontiguousarray(b1[sl].reshape(2, 128).T),
        })

    res = run_bass_kernel_spmd(
        nc, in_maps, core_ids=list(range(N_CORES)),
        trace=bool(_CACHE.get("trace", False)))
    _CACHE["last_results"] = res

    output = np.empty((B, L, EMBED), dtype=np.float32)
    for b in range(B):
        acc = res.results[b * HPC]["out"].astype(np.float32)
        acc[L // 2:] += res.results[b * HPC]["out2"].astype(np.float32)
        for hg in range(1, HPC):
            acc += res.results[b * HPC + hg]["out"].astype(np.float32)
            acc[L // 2:] += res.results[b * HPC + hg]["out2"].astype(np.float32)
        output[b] = acc + b1[None, :]
    return output


# revision 38
# speedup vs baseline: 1.0067x; 1.0015x over previous
"""Multi-headed attention (B=2, L=2048, E=1024, H=16) on 8 trn2 cores.

Sharding: batch (2) x head-groups (4) -> 8 cores. Each core computes 4 heads
of one batch element end-to-end (QKV projection, attention, partial output
projection); host sums the per-head-group partial outputs (out + out2) per
batch and adds the final bias.

All data is fp16 (host converts); PSUM accumulates fp32. Design, driven by
the TimelineSim cost model (matmul cost = out free size; exp is ACT-only):
  - QKV projections: W-slice stationary, x^T moving (N=512 chains).
  - Scores S^T [k, q] per head into [128,1024] PSUM, one wide exp per
    (head, k-tile) on ACT straight to fp16 SBUF (1038ns each; ACT busy
    ~134us is the co-critical path with PE ~140us).
  - PV flipped to out[q, d]: pt (exp'd scores) stationary, V [k, 64+1]
    moving (N=65; ones column accumulates the softmax denominator) -- half
    the PE rows of the N=q orientation.
  - Normalization fused into the ctx eviction (reciprocal + 0-stride
    broadcast multiply, batched per PSUM bank); ctx^T via DMA transpose
    (mid-kernel) or PE transpose (final pass, avoids DMA latency).
  - Attention runs in 4 passes of 2 heads ordered (qcp0,g0),(qcp1,g0),
    (qcp0,g1),(qcp1,g1) so passes 1-2 reuse g0 K/V and all g1 projection
    chains spread into later passes, balancing PE load per pass.
  - PSUM (8 banks): scores 2x[128,1024] (4) + ctx regions 130-wide pairs
    (3) + misc (1); one start/stop per bank per pass (zero-region rule).
  - Emission is software-pipelined: scores/exp run ~2 chunks ahead of PV;
    projection chains weave in as ~3-matmul micro-pieces; out-projection:
    qcp0 whole octiles in pass 4, qcp1 g0-part DMA'd to out2 during pass 3
    (host adds), g1-part at the tail woven with per-lt PE transposes.
"""

import numpy as np

EMBED = 1024
HEADS = 16
HD = 64
B = 2
L = 2048
N_CORES = 8
HPC = 4              # heads per core
ES = HPC * HD        # 256: e-slice width per core
NEC = EMBED // 128   # 8 embed chunks
NKT = L // 128       # 16 k-tiles

_CACHE = {}
EMITLOG = []


def _gen_kernel():
    from contextlib import ExitStack

    import concourse.mybir as mybir
    import concourse.tile as tile
    from concourse import bacc
    from concourse.masks import make_identity

    dt = mybir.dt
    f32 = dt.float32
    f16 = dt.float16

    nc = bacc.Bacc("TRN2", target_bir_lowering=False)

    xqT = nc.dram_tensor("xqT", [EMBED, L], f16, kind="ExternalInput")
    xkT = nc.dram_tensor("xkT", [EMBED, L], f16, kind="ExternalInput")
    xvT = nc.dram_tensor("xvT", [EMBED, L], f16, kind="ExternalInput")
    wT = nc.dram_tensor("wT", [EMBED, ES], f16, kind="ExternalInput")
    woT = nc.dram_tensor("woT", [ES, EMBED], f16, kind="ExternalInput")
    bqkv = nc.dram_tensor("bqkv", [128, 2], f32, kind="ExternalInput")
    out = nc.dram_tensor("out", [L, EMBED], f16, kind="ExternalOutput")
    out2 = nc.dram_tensor("out2", [L // 2, EMBED], f16, kind="ExternalOutput")

    # ctx region offset: (q8, hh) -> free offset in the 3-bank ctx tile.
    # Pairs of 130 (two heads x 65), 3 pairs per 512-f32 bank, no straddle.
    def ctx_off(q8, hh):
        return (q8 // 3) * 512 + (q8 % 3) * 130 + hh * 65

    with tile.TileContext(nc) as tc, ExitStack() as ctx:
        const = ctx.enter_context(tc.tile_pool(name="const", bufs=1))
        xin = ctx.enter_context(tc.tile_pool(name="xin", bufs=1))
        big = ctx.enter_context(tc.tile_pool(name="big", bufs=1))
        work = ctx.enter_context(tc.tile_pool(name="work", bufs=2))
        # PSUM budget (8 banks): scores 2x2 + ctx 3 + misc 1
        pp_s = ctx.enter_context(tc.tile_pool(name="pp_s", bufs=2, space="PSUM"))
        pp_ctx = ctx.enter_context(tc.tile_pool(name="pp_ctx", bufs=1, space="PSUM"))
        pp_m = ctx.enter_context(tc.tile_pool(name="pp_m", bufs=1, space="PSUM"))

        # ---- constants (g0 half of wt first; wo deferred) ------------------
        wt = const.tile([128, NEC * ES], f16)
        nc.sync.dma_start(
            wt[:].rearrange("p (c e) -> p c e", c=NEC),
            wT[:].rearrange("(c p) e -> p c e", p=128))
        bq = const.tile([128, 2], f32)
        nc.sync.dma_start(bq[:], bqkv[:])

        idn = const.tile([128, 128], f16)
        make_identity(nc, idn[:])

        # ---- x staging (fp16; xq chunk-tiles, xk/xv single tiles) ----------
        xq_sb = xin.tile([128, NEC * L], f16, name="xq")
        xk_sb = xin.tile([128, NEC * L], f16, name="xk")
        xv_sb = xin.tile([128, NEC * L], f16, name="xv")

        def xq_ap(c, qc):
            return xq_sb[:, c * L + qc * 512:(c * L) + (qc + 1) * 512]

        def xk_ap(c, qc):
            return xk_sb[:, c * L + qc * 512:(c * L) + (qc + 1) * 512]

        def xv_ap(c, qc):
            return xv_sb[:, c * L + qc * 512:(c * L) + (qc + 1) * 512]

        def big_dma(xsb, xdram, lo, hi):
            nc.sync.dma_start(
                xsb[:].rearrange("p (c l) -> p c l", l=L)[:, :, lo:hi],
                xdram[:].rearrange("(c p) l -> p c l", p=128)[:, :, lo:hi])

        # startup-critical order: q-chains overlap the xk-q0 transfer
        big_dma(xq_sb, xqT, 0, 512)
        big_dma(xq_sb, xqT, 512, 1024)
        big_dma(xk_sb, xkT, 0, 512)
        big_dma(xv_sb, xvT, 0, 512)
        big_dma(xk_sb, xkT, 512, 1024)
        big_dma(xv_sb, xvT, 512, 1024)
        big_dma(xk_sb, xkT, 1024, 2048)
        big_dma(xv_sb, xvT, 1024, 2048)

        wo = const.tile([128, 2 * EMBED], f16)
        nc.sync.dma_start(
            wo[:].rearrange("p (g e) -> p g e", g=2),
            woT[:].rearrange("(g p) e -> p g e", p=128),
        )

        big_dma(xq_sb, xqT, 1024, 2048)

        # ---- persistent activations ---------------------------------------
        # qt[qcp]: [e-group g partitions (2 heads x 64d), free g*1024 + q]
        qt = [big.tile([128, 2048], f16, name=f"qt{i}") for i in range(2)]
        # ktt[qc]: [d partitions, g*512 + k-local]
        ktt = [big.tile([128, 1024], f16, name=f"ktt{i}") for i in range(4)]
        # vaug[kt]: [k partitions, 4h x (64d + ones)]
        vaug = [big.tile([128, 4 * 65], f16, name=f"vaug{i}") for i in range(NKT)]
        # ctxT[qcp][g]: [c partitions (2 heads x 64d), q 1024]
        ctxT = [[big.tile([128, 1024], f16, name=f"ctxT{i}{g}") for g in range(2)]
                for i in range(2)]

        # ones columns of vaug (denominator trick), one strided memset per kt
        for kt in range(NKT):
            nc.gpsimd.memset(
                vaug[kt][:].rearrange("p (h x) -> p x h", x=65)[:, 64:65, :], 1.0)

        # ---- PE warmup during the DMA-bound prologue -----------------------
        warm = pp_m.tile([128, 512], f32, tag="m")
        for i in range(24):
            nc.tensor.matmul(
                warm[:, 0:128], lhsT=idn[:], rhs=idn[:],
                start=(i == 0), stop=(i == 23))

        # ---- projection chains (as micro-piece closures) --------------------
        def chain_pieces(xap, qc, g, write):
            """Returns piece closures: [3 mms], [3 mms], [2 mms + evict]."""
            state = {}

            def mms(c0, c1):
                def _p():
                    if c0 == 0:
                        state["ps"] = pp_m.tile([128, 512], f32, tag="m",
                                                name="ps")
                    ps = state["ps"]
                    for c in range(c0, c1):
                        nc.tensor.matmul(
                            ps[:],
                            lhsT=wt[:, c * ES + g * 128: c * ES + (g + 1) * 128],
                            rhs=xap(c, qc),
                            start=(c == 0), stop=(c == NEC - 1))
                    if c1 == NEC:
                        write(state["ps"])
                return _p

            return [mms(0, 3), mms(3, 6), mms(6, 8)]

        def chain_q(qc, g):
            def write(ps):
                nc.vector.tensor_scalar_add(
                    qt[qc // 2][:, g * 1024 + (qc % 2) * 512:
                                g * 1024 + (qc % 2 + 1) * 512],
                    ps[:], bq[:, g:g + 1])
            return chain_pieces(xq_ap, qc, g, write)

        def chain_k(qc, g):
            def write(ps):
                nc.vector.tensor_scalar_add(
                    ktt[qc][:, g * 512:(g + 1) * 512], ps[:], bq[:, g:g + 1])
            return chain_pieces(xk_ap, qc, g, write)

        def chain_v(qc, g):
            """k/q pieces plus V transpose pieces into vaug."""
            state = {}

            def write(ps):
                vtt = work.tile([128, 512], f16, tag="vtt", bufs=2)
                nc.vector.tensor_scalar_add(vtt[:], ps[:], bq[:, g:g + 1])
                state["vtt"] = vtt

            pieces = chain_pieces(xv_ap, qc, g, write)

            def tp_piece(j0):
                def _p():
                    vtt = state["vtt"]
                    for j in (j0, j0 + 1):
                        tp = pp_m.tile([128, 128], f16, tag="m", name="tp")
                        nc.tensor.transpose(
                            tp[:], vtt[:, j * 128:(j + 1) * 128], idn[:])
                        nc.vector.tensor_copy(
                            vaug[qc * 4 + j][:, (2 * g) * 65:(2 * g + 2) * 65]
                            .rearrange("p (h x) -> p h x", h=2)[:, :, 0:64],
                            tp[:].rearrange("p (h x) -> p h x", h=2))
                return _p

            return pieces + [tp_piece(0), tp_piece(2)]

        # ---- attention pass: one e-group (2 heads) over one qcp ------------
        inv_sqrt_e = 1.0 / 32.0

        def make_pass(qcp, g, last=False):
            """Returns (S, P): scores/exp chunks and pv/finalize chunks."""
            state = {}

            def scores_chunk(kt, hh):
                sps = pp_s.tile([128, 1024], f32, tag="s")
                off = hh * 64
                for half in range(2):
                    nc.tensor.matmul(
                        sps[:, half * 512:(half + 1) * 512],
                        lhsT=ktt[kt // 4][
                            off:off + 64,
                            g * 512 + (kt % 4) * 128: g * 512 + (kt % 4 + 1) * 128],
                        rhs=qt[qcp][off:off + 64,
                                    g * 1024 + half * 512: g * 1024 + (half + 1) * 512],
                        start=True, stop=True)
                pt = work.tile([128, 1024], f16, tag="pt", bufs=6)
                nc.scalar.activation(
                    pt[:], sps[:], mybir.ActivationFunctionType.Exp,
                    scale=inv_sqrt_e)
                state[("pt", kt, hh)] = pt

            def pv_chunk(kt, hh):
                # one PSUM start/stop per bank per pass (zero-region rule):
                # bank b of ctx starts at (kt0, hh0, q8=3b), stops at
                # (kt15, hh1, q8 = 2/5/7).
                if kt == 0 and hh == 0:
                    state["ctx"] = pp_ctx.tile(
                        [128, 1536], f32, tag="ctx", name="ctxp")
                ctxp = state["ctx"]
                pt = state.pop(("pt", kt, hh))
                h = 2 * g + hh
                for q8 in range(8):
                    o = ctx_off(q8, hh)
                    nc.tensor.matmul(
                        ctxp[:, o: o + 65],
                        lhsT=pt[:, q8 * 128:(q8 + 1) * 128],
                        rhs=vaug[kt][:, h * 65:(h + 1) * 65],
                        start=(kt == 0 and hh == 0 and q8 % 3 == 0),
                        stop=(kt == NKT - 1 and hh == 1 and q8 in (2, 5, 7)))

            def fin_batch():
                """Batched finalize: 3 per-bank recips + 3 per-bank muls
                (rec broadcast via 0-stride), then 8 async DMA transposes
                into ctxT. Short critical chain at the pass boundary."""
                ctxp = state["ctx"]
                rec = work.tile([128, 16], f32, tag="rec", bufs=2, name="rec")
                cn = work.tile([128, 1024], f16, tag="cn", bufs=2, name="cn")
                for b in range(3):
                    npair = 3 if b < 2 else 2
                    den_view = ctxp[:, b * 512: b * 512 + npair * 130].rearrange(
                        "p (r h x) -> p r h x", h=2, x=65)[:, :, :, 64:65]
                    nc.vector.reciprocal(
                        rec[:, b * 6: b * 6 + npair * 2].rearrange(
                            "p (r h x) -> p r h x", h=2, x=1),
                        den_view)
                for b in range(3):
                    npair = 3 if b < 2 else 2
                    nc.vector.tensor_mul(
                        cn[:, b * 3 * 128: (b * 3 + npair) * 128].rearrange(
                            "p (r h x) -> p r h x", h=2, x=64),
                        ctxp[:, b * 512: b * 512 + npair * 130].rearrange(
                            "p (r h x) -> p r h x", h=2, x=65)[:, :, :, 0:64],
                        rec[:, b * 6: b * 6 + npair * 2].rearrange(
                            "p (r h x) -> p r h x", h=2, x=1).broadcast_to(
                            [128, npair, 2, 64]))
                if last:
                    _CACHE["last_cn"] = cn
                    return
                for q8 in range(8):
                    nc.sync.dma_start_transpose(
                        ctxT[qcp][g][:, q8 * 128:(q8 + 1) * 128],
                        cn[:, q8 * 128:(q8 + 1) * 128])

            S = [(lambda kt=kt, hh=hh: scores_chunk(kt, hh))
                 for kt in range(NKT) for hh in range(2)]
            P = [("pv", (lambda kt=kt, hh=hh: pv_chunk(kt, hh)))
                 for kt in range(NKT) for hh in range(2)]
            P += [("fin", fin_batch)]
            return S, P

        # ---- output projection ----------------------------------------------
        def make_outproj(qcp, pool_tags, evict_split=False):
            """Yields single-matmul pieces: (g0 mm) then (g1 mm + evict)."""
            state = {}

            def part(lt, oc, g, slot_i):
                if g == 0:
                    pool, tag = pool_tags[slot_i % len(pool_tags)]
                    state["ops"] = pool.tile([128, 512], f32, tag=tag,
                                             name="ops")
                ops = state["ops"]
                nc.tensor.matmul(
                    ops[:],
                    lhsT=ctxT[qcp][g][:, lt * 128:(lt + 1) * 128],
                    rhs=wo[:, g * EMBED + oc * 512: g * EMBED + (oc + 1) * 512],
                    start=(g == 0), stop=(g == 1))
                if g == 1:
                    if oc == 0:
                        state["ot"] = work.tile([128, 1024], f16, tag="ot",
                                                bufs=4, name="ot")
                    ot = state["ot"]
                    if evict_split and slot_i % 2 == 1:
                        nc.scalar.copy(ot[:, oc * 512:(oc + 1) * 512], ops[:])
                    else:
                        nc.vector.tensor_copy(
                            ot[:, oc * 512:(oc + 1) * 512], ops[:])
                    if oc == 1:
                        lt_g = qcp * 8 + lt
                        nc.sync.dma_start(
                            out[lt_g * 128:(lt_g + 1) * 128, :], ot[:])

            i = 0
            for lt in range(8):
                for oc in range(2):
                    for g in range(2):
                        yield (lambda lt=lt, oc=oc, g=g, i=i: part(lt, oc, g, i))
                    i += 1

        # qcp1 g-split: g0 partials evicted to ot0 staging during pass 3,
        # g1 matmuls + adds + DMA at the tail.
        ot0 = [work.tile([128, 1024], f16, tag="ot0", bufs=8, name=f"ot0_{lt}")
               for lt in range(8)]

        def make_op_g0(qcp, pool_tags):
            def piece(lt, oc, slot_i):
                pool, tag = pool_tags[slot_i % len(pool_tags)]
                ops = pool.tile([128, 512], f32, tag=tag, name="ops0")
                nc.tensor.matmul(
                    ops[:],
                    lhsT=ctxT[qcp][0][:, lt * 128:(lt + 1) * 128],
                    rhs=wo[:, oc * 512:(oc + 1) * 512],
                    start=True, stop=True)
                nc.vector.tensor_copy(ot0[lt][:, oc * 512:(oc + 1) * 512],
                                      ops[:])
                if oc == 1:
                    nc.sync.dma_start(out2[lt * 128:(lt + 1) * 128, :],
                                      ot0[lt][:])

            i = 0
            for lt in range(8):
                for oc in range(2):
                    yield (lambda lt=lt, oc=oc, i=i: piece(lt, oc, i))
                    i += 1

        def make_op_g1(qcp, pool_tags):
            state = {}

            def piece(lt, oc, slot_i):
                pool, tag = pool_tags[slot_i % len(pool_tags)]
                ops = pool.tile([128, 512], f32, tag=tag, name="ops1")
                nc.tensor.matmul(
                    ops[:],
                    lhsT=ctxT[qcp][1][:, lt * 128:(lt + 1) * 128],
                    rhs=wo[:, EMBED + oc * 512: EMBED + (oc + 1) * 512],
                    start=True, stop=True)
                if oc == 0:
                    state["ot"] = work.tile([128, 1024], f16, tag="ot",
                                            bufs=4, name="ot")
                ot = state["ot"]
                if slot_i % 2 == 1:
                    nc.scalar.copy(ot[:, oc * 512:(oc + 1) * 512], ops[:])
                else:
                    nc.vector.tensor_copy(ot[:, oc * 512:(oc + 1) * 512],
                                          ops[:])
                if oc == 1:
                    lt_g = qcp * 8 + lt
                    nc.sync.dma_start(
                        out[lt_g * 128:(lt_g + 1) * 128, :], ot[:])

            i = 0
            for lt in range(8):
                for oc in range(2):
                    yield (lambda lt=lt, oc=oc, i=i: piece(lt, oc, i))
                    i += 1

        # ---- emission schedule ----------------------------------------------
        # Pass order (0,g0),(1,g0),(0,g1),(1,g1): passes 1-2 share the g0
        # K/V tensors so all g1 projection chains defer to later passes,
        # balancing PE load against the ACT-bound exp stream in every pass.
        # interleave k00/q00 chain pieces so both track their DMAs; q00/q10
        # use the (still free) scores psum slots to run parallel to k00
        def chain_q_slot(qc):
            ps = pp_s.tile([128, 512], f32, tag="s", name="psq")
            for c in range(NEC):
                nc.tensor.matmul(
                    ps[:],
                    lhsT=wt[:, c * ES: c * ES + 128],
                    rhs=xq_ap(c, qc),
                    start=(c == 0), stop=(c == NEC - 1))
            nc.vector.tensor_scalar_add(
                qt[0][:, qc * 512:(qc + 1) * 512], ps[:], bq[:, 0:1])

        chain_q_slot(0)
        chain_q_slot(1)
        for piece in chain_k(0, 0):
            piece()


        S_all, P_all = [], []
        for (qcp, g) in [(0, 0), (1, 0), (0, 1), (1, 1)]:
            S, P = make_pass(qcp, g, last=(qcp == 1 and g == 1))
            S_all += S
            P_all += P

        def log(label):
            nm = nc.get_next_instruction_name()  # peeks+consumes one id
            EMITLOG.append((label, int(nm.split("-")[1])))

        S_all[0]()
        log("S0")
        S_all[1]()
        log("S1")
        for piece in chain_v(0, 0):
            piece()
        log("v00")

        from collections import deque
        pieces = deque()
        for cl in (chain_k(1, 0), chain_v(1, 0),
                   chain_k(2, 0), chain_k(3, 0), chain_v(2, 0), chain_v(3, 0),
                   chain_q(2, 0), chain_q(3, 0),
                   chain_k(0, 1), chain_k(1, 1), chain_k(2, 1), chain_k(3, 1),
                   chain_v(0, 1), chain_v(1, 1),
                   chain_q(0, 1), chain_q(1, 1),
                   chain_v(2, 1), chain_v(3, 1),
                   chain_q(2, 1), chain_q(3, 1)):
            pieces.extend(cl)

        def drain(n):
            for _ in range(n):
                if pieces:
                    pieces.popleft()()

        # P-step indexing: pass p occupies [33p, 33p+32]; 32 pv + 1 fin batch.
        # op-C (qcp1 g0 partials) woven into pass 3; op0 (qcp0, full octiles)
        # into pass 4; op-D (qcp1 g1 + adds) at the tail.
        opC = list(make_op_g0(1, [(pp_m, "m")]))
        opC_at = {67 + 2 * j: cl for j, cl in enumerate(opC)}
        op0 = list(make_outproj(0, [(pp_m, "m")]))
        op0_at = {100 + j: cl for j, cl in enumerate(op0)}

        si = 2
        pv_done = 0
        for pi, (kind, p) in enumerate(P_all):
            p()
            log(f"P{pi}:{kind}")
            if kind == "pv":
                pv_done += 1
            if pi in opC_at:
                opC_at[pi]()
                log(f"opC@{pi}")
            if pi in op0_at:
                op0_at[pi]()
                log(f"op0@{pi}")
            if kind == "pv":
                drain(2 if pi < 4 else 1)
                log(f"drain@{pi}")
            while si < len(S_all) and si < pv_done + 5:
                S_all[si]()
                log(f"S{si}")
                si += 1
        while si < len(S_all):
            S_all[si]()
            si += 1
        drain(len(pieces))
        log("tail-start")

        # tail: per-lt, PE-transpose the last pass's cn slice into ctxT,
        # immediately followed by that lt's op-D pieces.
        cn_last = _CACHE.pop("last_cn")
        opD = list(make_op_g1(1, [(pp_s, "s"), (pp_s, "s"), (pp_ctx, "ctx")]))
        for lt in range(8):
            tp = pp_m.tile([128, 128], f16, tag="m", name="tpd")
            nc.tensor.transpose(
                tp[:], cn_last[:, lt * 128:(lt + 1) * 128], idn[:])
            nc.vector.tensor_copy(ctxT[1][1][:, lt * 128:(lt + 1) * 128], tp[:])
            opD[2 * lt]()
            opD[2 * lt + 1]()

    nc.compile()
    return nc


def kernel(query, key, values, W1, b1):
    from concourse.bass_utils import run_bass_kernel_spmd

    if "nc" not in _CACHE:
        _CACHE["nc"] = _gen_kernel()
    nc = _CACHE["nc"]

    query = np.asarray(query, dtype=np.float32)
    key = np.asarray(key, dtype=np.float32)
    values = np.asarray(values, dtype=np.float32)
    W1 = np.asarray(W1, dtype=np.float32)
    b1 = np.asarray(b1, dtype=np.float32)

    xT = {}
    for b in range(B):
        xT[("q", b)] = np.ascontiguousarray(query[b].T).astype(np.float16)
        xT[("k", b)] = np.ascontiguousarray(key[b].T).astype(np.float16)
        xT[("v", b)] = np.ascontiguousarray(values[b].T).astype(np.float16)

    in_maps = []
    for core in range(N_CORES):
        b = core // HPC
        hg = core % HPC
        sl = slice(hg * ES, (hg + 1) * ES)
        in_maps.append({
            "xqT": xT[("q", b)],
            "xkT": xT[("k", b)],
            "xvT": xT[("v", b)],
            "wT": np.ascontiguousarray(W1[sl, :].T).astype(np.float16),
            "woT": np.ascontiguousarray(W1[:, sl].T).astype(np.float16),
            "bqkv": np.ascontiguousarray(b1[sl].reshape(2, 128).T),
        })

    res = run_bass_kernel_spmd(
        nc, in_maps, core_ids=list(range(N_CORES)),
        trace=bool(_CACHE.get("trace", False)))
    _CACHE["last_results"] = res

    output = np.empty((B, L, EMBED), dtype=np.float32)
    for b in range(B):
        acc = res.results[b * HPC]["out"].astype(np.float32)
        acc[L // 2:] += res.results[b * HPC]["out2"].astype(np.float32)
        for hg in range(1, HPC):
            acc += res.results[b * HPC + hg]["out"].astype(np.float32)
            acc[L // 2:] += res.results[b * HPC + hg]["out2"].astype(np.float32)
        output[b] = acc + b1[None, :]
    return output


# revision 39
# speedup vs baseline: 1.0079x; 1.0012x over previous
"""Multi-headed attention (B=2, L=2048, E=1024, H=16) on 8 trn2 cores.

Sharding: batch (2) x head-groups (4) -> 8 cores. Each core computes 4 heads
of one batch element end-to-end (QKV projection, attention, partial output
projection); host sums the per-head-group partial outputs (out + out2) per
batch and adds the final bias.

All data is fp16 (host converts); PSUM accumulates fp32. Design, driven by
the TimelineSim cost model (matmul cost = out free size; exp is ACT-only):
  - QKV projections: W-slice stationary, x^T moving (N=512 chains).
  - Scores S^T [k, q] per head into [128,1024] PSUM, one wide exp per
    (head, k-tile) on ACT straight to fp16 SBUF (1038ns each; ACT busy
    ~134us is the co-critical path with PE ~140us).
  - PV flipped to out[q, d]: pt (exp'd scores) stationary, V [k, 64+1]
    moving (N=65; ones column accumulates the softmax denominator) -- half
    the PE rows of the N=q orientation.
  - Normalization fused into the ctx eviction (reciprocal + 0-stride
    broadcast multiply, batched per PSUM bank); ctx^T via DMA transpose
    (mid-kernel) or PE transpose (final pass, avoids DMA latency).
  - Attention runs in 4 passes of 2 heads ordered (qcp0,g0),(qcp1,g0),
    (qcp0,g1),(qcp1,g1) so passes 1-2 reuse g0 K/V and all g1 projection
    chains spread into later passes, balancing PE load per pass.
  - PSUM (8 banks): scores 2x[128,1024] (4) + ctx regions 130-wide pairs
    (3) + misc (1); one start/stop per bank per pass (zero-region rule).
  - Emission is software-pipelined: scores/exp run ~2 chunks ahead of PV;
    projection chains weave in as ~3-matmul micro-pieces; out-projection:
    qcp0 whole octiles in pass 4, qcp1 g0-part DMA'd to out2 during pass 3
    (host adds), g1-part at the tail woven with per-lt PE transposes.
"""

import numpy as np

EMBED = 1024
HEADS = 16
HD = 64
B = 2
L = 2048
N_CORES = 8
HPC = 4              # heads per core
ES = HPC * HD        # 256: e-slice width per core
NEC = EMBED // 128   # 8 embed chunks
NKT = L // 128       # 16 k-tiles

_CACHE = {}
EMITLOG = []


def _gen_kernel():
    from contextlib import ExitStack

    import concourse.mybir as mybir
    import concourse.tile as tile
    from concourse import bacc
    from concourse.masks import make_identity

    dt = mybir.dt
    f32 = dt.float32
    f16 = dt.float16

    nc = bacc.Bacc("TRN2", target_bir_lowering=False)

    xqT = nc.dram_tensor("xqT", [EMBED, L], f16, kind="ExternalInput")
    xkT = nc.dram_tensor("xkT", [EMBED, L], f16, kind="ExternalInput")
    xvT = nc.dram_tensor("xvT", [EMBED, L], f16, kind="ExternalInput")
    wT = nc.dram_tensor("wT", [EMBED, ES], f16, kind="ExternalInput")
    woT = nc.dram_tensor("woT", [ES, EMBED], f16, kind="ExternalInput")
    bqkv = nc.dram_tensor("bqkv", [128, 2], f32, kind="ExternalInput")
    out = nc.dram_tensor("out", [L, EMBED], f16, kind="ExternalOutput")
    out2 = nc.dram_tensor("out2", [L // 2, EMBED], f16, kind="ExternalOutput")

    # ctx region offset: (q8, hh) -> free offset in the 3-bank ctx tile.
    # Pairs of 130 (two heads x 65), 3 pairs per 512-f32 bank, no straddle.
    def ctx_off(q8, hh):
        return (q8 // 3) * 512 + (q8 % 3) * 130 + hh * 65

    with tile.TileContext(nc) as tc, ExitStack() as ctx:
        const = ctx.enter_context(tc.tile_pool(name="const", bufs=1))
        xin = ctx.enter_context(tc.tile_pool(name="xin", bufs=1))
        big = ctx.enter_context(tc.tile_pool(name="big", bufs=1))
        work = ctx.enter_context(tc.tile_pool(name="work", bufs=2))
        # PSUM budget (8 banks): scores 2x2 + ctx 3 + misc 1
        pp_s = ctx.enter_context(tc.tile_pool(name="pp_s", bufs=2, space="PSUM"))
        pp_ctx = ctx.enter_context(tc.tile_pool(name="pp_ctx", bufs=1, space="PSUM"))
        pp_m = ctx.enter_context(tc.tile_pool(name="pp_m", bufs=1, space="PSUM"))

        # ---- constants (g0 half of wt first; wo deferred) ------------------
        wt = const.tile([128, NEC * ES], f16)
        nc.sync.dma_start(
            wt[:].rearrange("p (c e) -> p c e", c=NEC),
            wT[:].rearrange("(c p) e -> p c e", p=128))
        bq = const.tile([128, 2], f32)
        nc.sync.dma_start(bq[:], bqkv[:])

        idn = const.tile([128, 128], f16)
        make_identity(nc, idn[:])

        # ---- x staging (fp16; xq chunk-tiles, xk/xv single tiles) ----------
        xq_sb = xin.tile([128, NEC * L], f16, name="xq")
        xk_sb = xin.tile([128, NEC * L], f16, name="xk")
        xv_sb = xin.tile([128, NEC * L], f16, name="xv")

        def xq_ap(c, qc):
            return xq_sb[:, c * L + qc * 512:(c * L) + (qc + 1) * 512]

        def xk_ap(c, qc):
            return xk_sb[:, c * L + qc * 512:(c * L) + (qc + 1) * 512]

        def xv_ap(c, qc):
            return xv_sb[:, c * L + qc * 512:(c * L) + (qc + 1) * 512]

        def big_dma(xsb, xdram, lo, hi):
            nc.sync.dma_start(
                xsb[:].rearrange("p (c l) -> p c l", l=L)[:, :, lo:hi],
                xdram[:].rearrange("(c p) l -> p c l", p=128)[:, :, lo:hi])

        # startup-critical order: xk-q0 lands between the xq quarters so
        # k00 overlaps q10; q10 is only needed by S0's second matmul
        big_dma(xq_sb, xqT, 0, 512)
        big_dma(xk_sb, xkT, 0, 512)
        big_dma(xq_sb, xqT, 512, 1024)
        big_dma(xv_sb, xvT, 0, 512)
        big_dma(xk_sb, xkT, 512, 1024)
        big_dma(xv_sb, xvT, 512, 1024)
        big_dma(xk_sb, xkT, 1024, 2048)
        big_dma(xv_sb, xvT, 1024, 2048)

        wo = const.tile([128, 2 * EMBED], f16)
        nc.sync.dma_start(
            wo[:].rearrange("p (g e) -> p g e", g=2),
            woT[:].rearrange("(g p) e -> p g e", p=128),
        )

        big_dma(xq_sb, xqT, 1024, 2048)

        # ---- persistent activations ---------------------------------------
        # qt[qcp]: [e-group g partitions (2 heads x 64d), free g*1024 + q]
        qt = [big.tile([128, 2048], f16, name=f"qt{i}") for i in range(2)]
        # ktt[qc]: [d partitions, g*512 + k-local]
        ktt = [big.tile([128, 1024], f16, name=f"ktt{i}") for i in range(4)]
        # vaug[kt]: [k partitions, 4h x (64d + ones)]
        vaug = [big.tile([128, 4 * 65], f16, name=f"vaug{i}") for i in range(NKT)]
        # ctxT[qcp][g]: [c partitions (2 heads x 64d), q 1024]
        ctxT = [[big.tile([128, 1024], f16, name=f"ctxT{i}{g}") for g in range(2)]
                for i in range(2)]

        # ones columns of vaug (denominator trick), one strided memset per kt
        for kt in range(NKT):
            nc.gpsimd.memset(
                vaug[kt][:].rearrange("p (h x) -> p x h", x=65)[:, 64:65, :], 1.0)

        # ---- PE warmup during the DMA-bound prologue -----------------------
        warm = pp_m.tile([128, 512], f32, tag="m")
        for i in range(24):
            nc.tensor.matmul(
                warm[:, 0:128], lhsT=idn[:], rhs=idn[:],
                start=(i == 0), stop=(i == 23))

        # ---- projection chains (as micro-piece closures) --------------------
        def chain_pieces(xap, qc, g, write):
            """Returns piece closures: [3 mms], [3 mms], [2 mms + evict]."""
            state = {}

            def mms(c0, c1):
                def _p():
                    if c0 == 0:
                        state["ps"] = pp_m.tile([128, 512], f32, tag="m",
                                                name="ps")
                    ps = state["ps"]
                    for c in range(c0, c1):
                        nc.tensor.matmul(
                            ps[:],
                            lhsT=wt[:, c * ES + g * 128: c * ES + (g + 1) * 128],
                            rhs=xap(c, qc),
                            start=(c == 0), stop=(c == NEC - 1))
                    if c1 == NEC:
                        write(state["ps"])
                return _p

            return [mms(0, 3), mms(3, 6), mms(6, 8)]

        def chain_q(qc, g):
            def write(ps):
                nc.vector.tensor_scalar_add(
                    qt[qc // 2][:, g * 1024 + (qc % 2) * 512:
                                g * 1024 + (qc % 2 + 1) * 512],
                    ps[:], bq[:, g:g + 1])
            return chain_pieces(xq_ap, qc, g, write)

        def chain_k(qc, g):
            def write(ps):
                nc.vector.tensor_scalar_add(
                    ktt[qc][:, g * 512:(g + 1) * 512], ps[:], bq[:, g:g + 1])
            return chain_pieces(xk_ap, qc, g, write)

        def chain_v(qc, g):
            """k/q pieces plus V transpose pieces into vaug."""
            state = {}

            def write(ps):
                vtt = work.tile([128, 512], f16, tag="vtt", bufs=2)
                nc.vector.tensor_scalar_add(vtt[:], ps[:], bq[:, g:g + 1])
                state["vtt"] = vtt

            pieces = chain_pieces(xv_ap, qc, g, write)

            def tp_piece(j0):
                def _p():
                    vtt = state["vtt"]
                    for j in (j0, j0 + 1):
                        tp = pp_m.tile([128, 128], f16, tag="m", name="tp")
                        nc.tensor.transpose(
                            tp[:], vtt[:, j * 128:(j + 1) * 128], idn[:])
                        nc.vector.tensor_copy(
                            vaug[qc * 4 + j][:, (2 * g) * 65:(2 * g + 2) * 65]
                            .rearrange("p (h x) -> p h x", h=2)[:, :, 0:64],
                            tp[:].rearrange("p (h x) -> p h x", h=2))
                return _p

            return pieces + [tp_piece(0), tp_piece(2)]

        # ---- attention pass: one e-group (2 heads) over one qcp ------------
        inv_sqrt_e = 1.0 / 32.0

        def make_pass(qcp, g, last=False):
            """Returns (S, P): scores/exp chunks and pv/finalize chunks."""
            state = {}

            def scores_chunk(kt, hh):
                sps = pp_s.tile([128, 1024], f32, tag="s")
                off = hh * 64
                for half in range(2):
                    nc.tensor.matmul(
                        sps[:, half * 512:(half + 1) * 512],
                        lhsT=ktt[kt // 4][
                            off:off + 64,
                            g * 512 + (kt % 4) * 128: g * 512 + (kt % 4 + 1) * 128],
                        rhs=qt[qcp][off:off + 64,
                                    g * 1024 + half * 512: g * 1024 + (half + 1) * 512],
                        start=True, stop=True)
                pt = work.tile([128, 1024], f16, tag="pt", bufs=6)
                nc.scalar.activation(
                    pt[:], sps[:], mybir.ActivationFunctionType.Exp,
                    scale=inv_sqrt_e)
                state[("pt", kt, hh)] = pt

            def pv_chunk(kt, hh):
                # one PSUM start/stop per bank per pass (zero-region rule):
                # bank b of ctx starts at (kt0, hh0, q8=3b), stops at
                # (kt15, hh1, q8 = 2/5/7).
                if kt == 0 and hh == 0:
                    state["ctx"] = pp_ctx.tile(
                        [128, 1536], f32, tag="ctx", name="ctxp")
                ctxp = state["ctx"]
                pt = state.pop(("pt", kt, hh))
                h = 2 * g + hh
                for q8 in range(8):
                    o = ctx_off(q8, hh)
                    nc.tensor.matmul(
                        ctxp[:, o: o + 65],
                        lhsT=pt[:, q8 * 128:(q8 + 1) * 128],
                        rhs=vaug[kt][:, h * 65:(h + 1) * 65],
                        start=(kt == 0 and hh == 0 and q8 % 3 == 0),
                        stop=(kt == NKT - 1 and hh == 1 and q8 in (2, 5, 7)))

            def fin_batch():
                """Batched finalize: 3 per-bank recips + 3 per-bank muls
                (rec broadcast via 0-stride), then 8 async DMA transposes
                into ctxT. Short critical chain at the pass boundary."""
                ctxp = state["ctx"]
                rec = work.tile([128, 16], f32, tag="rec", bufs=2, name="rec")
                cn = work.tile([128, 1024], f16, tag="cn", bufs=2, name="cn")
                for b in range(3):
                    npair = 3 if b < 2 else 2
                    den_view = ctxp[:, b * 512: b * 512 + npair * 130].rearrange(
                        "p (r h x) -> p r h x", h=2, x=65)[:, :, :, 64:65]
                    nc.vector.reciprocal(
                        rec[:, b * 6: b * 6 + npair * 2].rearrange(
                            "p (r h x) -> p r h x", h=2, x=1),
                        den_view)
                for b in range(3):
                    npair = 3 if b < 2 else 2
                    nc.vector.tensor_mul(
                        cn[:, b * 3 * 128: (b * 3 + npair) * 128].rearrange(
                            "p (r h x) -> p r h x", h=2, x=64),
                        ctxp[:, b * 512: b * 512 + npair * 130].rearrange(
                            "p (r h x) -> p r h x", h=2, x=65)[:, :, :, 0:64],
                        rec[:, b * 6: b * 6 + npair * 2].rearrange(
                            "p (r h x) -> p r h x", h=2, x=1).broadcast_to(
                            [128, npair, 2, 64]))
                if last:
                    _CACHE["last_cn"] = cn
                    return
                for q8 in range(8):
                    nc.sync.dma_start_transpose(
                        ctxT[qcp][g][:, q8 * 128:(q8 + 1) * 128],
                        cn[:, q8 * 128:(q8 + 1) * 128])

            S = [(lambda kt=kt, hh=hh: scores_chunk(kt, hh))
                 for kt in range(NKT) for hh in range(2)]
            P = [("pv", (lambda kt=kt, hh=hh: pv_chunk(kt, hh)))
                 for kt in range(NKT) for hh in range(2)]
            P += [("fin", fin_batch)]
            return S, P

        # ---- output projection ----------------------------------------------
        def make_outproj(qcp, pool_tags, evict_split=False):
            """Yields single-matmul pieces: (g0 mm) then (g1 mm + evict)."""
            state = {}

            def part(lt, oc, g, slot_i):
                if g == 0:
                    pool, tag = pool_tags[slot_i % len(pool_tags)]
                    state["ops"] = pool.tile([128, 512], f32, tag=tag,
                                             name="ops")
                ops = state["ops"]
                nc.tensor.matmul(
                    ops[:],
                    lhsT=ctxT[qcp][g][:, lt * 128:(lt + 1) * 128],
                    rhs=wo[:, g * EMBED + oc * 512: g * EMBED + (oc + 1) * 512],
                    start=(g == 0), stop=(g == 1))
                if g == 1:
                    if oc == 0:
                        state["ot"] = work.tile([128, 1024], f16, tag="ot",
                                                bufs=4, name="ot")
                    ot = state["ot"]
                    if evict_split and slot_i % 2 == 1:
                        nc.scalar.copy(ot[:, oc * 512:(oc + 1) * 512], ops[:])
                    else:
                        nc.vector.tensor_copy(
                            ot[:, oc * 512:(oc + 1) * 512], ops[:])
                    if oc == 1:
                        lt_g = qcp * 8 + lt
                        nc.sync.dma_start(
                            out[lt_g * 128:(lt_g + 1) * 128, :], ot[:])

            i = 0
            for lt in range(8):
                for oc in range(2):
                    for g in range(2):
                        yield (lambda lt=lt, oc=oc, g=g, i=i: part(lt, oc, g, i))
                    i += 1

        # qcp1 g-split: g0 partials evicted to ot0 staging during pass 3,
        # g1 matmuls + adds + DMA at the tail.
        ot0 = [work.tile([128, 1024], f16, tag="ot0", bufs=8, name=f"ot0_{lt}")
               for lt in range(8)]

        def make_op_g0(qcp, pool_tags):
            def piece(lt, oc, slot_i):
                pool, tag = pool_tags[slot_i % len(pool_tags)]
                ops = pool.tile([128, 512], f32, tag=tag, name="ops0")
                nc.tensor.matmul(
                    ops[:],
                    lhsT=ctxT[qcp][0][:, lt * 128:(lt + 1) * 128],
                    rhs=wo[:, oc * 512:(oc + 1) * 512],
                    start=True, stop=True)
                nc.vector.tensor_copy(ot0[lt][:, oc * 512:(oc + 1) * 512],
                                      ops[:])
                if oc == 1:
                    nc.sync.dma_start(out2[lt * 128:(lt + 1) * 128, :],
                                      ot0[lt][:])

            i = 0
            for lt in range(8):
                for oc in range(2):
                    yield (lambda lt=lt, oc=oc, i=i: piece(lt, oc, i))
                    i += 1

        def make_op_g1(qcp, pool_tags):
            state = {}

            def piece(lt, oc, slot_i):
                pool, tag = pool_tags[slot_i % len(pool_tags)]
                ops = pool.tile([128, 512], f32, tag=tag, name="ops1")
                nc.tensor.matmul(
                    ops[:],
                    lhsT=ctxT[qcp][1][:, lt * 128:(lt + 1) * 128],
                    rhs=wo[:, EMBED + oc * 512: EMBED + (oc + 1) * 512],
                    start=True, stop=True)
                if oc == 0:
                    state["ot"] = work.tile([128, 1024], f16, tag="ot",
                                            bufs=4, name="ot")
                ot = state["ot"]
                if slot_i % 2 == 1:
                    nc.scalar.copy(ot[:, oc * 512:(oc + 1) * 512], ops[:])
                else:
                    nc.vector.tensor_copy(ot[:, oc * 512:(oc + 1) * 512],
                                          ops[:])
                if oc == 1:
                    lt_g = qcp * 8 + lt
                    nc.sync.dma_start(
                        out[lt_g * 128:(lt_g + 1) * 128, :], ot[:])

            i = 0
            for lt in range(8):
                for oc in range(2):
                    yield (lambda lt=lt, oc=oc, i=i: piece(lt, oc, i))
                    i += 1

        # ---- emission schedule ----------------------------------------------
        # Pass order (0,g0),(1,g0),(0,g1),(1,g1): passes 1-2 share the g0
        # K/V tensors so all g1 projection chains defer to later passes,
        # balancing PE load against the ACT-bound exp stream in every pass.
        # interleave k00/q00 chain pieces so both track their DMAs; q00/q10
        # use the (still free) scores psum slots to run parallel to k00
        def chain_q_slot(qc):
            ps = pp_s.tile([128, 512], f32, tag="s", name="psq")
            for c in range(NEC):
                nc.tensor.matmul(
                    ps[:],
                    lhsT=wt[:, c * ES: c * ES + 128],
                    rhs=xq_ap(c, qc),
                    start=(c == 0), stop=(c == NEC - 1))
            nc.vector.tensor_scalar_add(
                qt[0][:, qc * 512:(qc + 1) * 512], ps[:], bq[:, 0:1])

        chain_q_slot(0)
        for piece in chain_k(0, 0):
            piece()
        chain_q_slot(1)


        S_all, P_all = [], []
        for (qcp, g) in [(0, 0), (1, 0), (0, 1), (1, 1)]:
            S, P = make_pass(qcp, g, last=(qcp == 1 and g == 1))
            S_all += S
            P_all += P

        def log(label):
            nm = nc.get_next_instruction_name()  # peeks+consumes one id
            EMITLOG.append((label, int(nm.split("-")[1])))

        S_all[0]()
        log("S0")
        S_all[1]()
        log("S1")
        for piece in chain_v(0, 0):
            piece()
        log("v00")

        from collections import deque
        pieces = deque()
        for cl in (chain_k(1, 0), chain_v(1, 0),
                   chain_k(2, 0), chain_k(3, 0), chain_v(2, 0), chain_v(3, 0),
                   chain_q(2, 0), chain_q(3, 0),
                   chain_k(0, 1), chain_k(1, 1), chain_k(2, 1), chain_k(3, 1),
                   chain_v(0, 1), chain_v(1, 1),
                   chain_q(0, 1), chain_q(1, 1),
                   chain_v(2, 1), chain_v(3, 1),
                   chain_q(2, 1), chain_q(3, 1)):
            pieces.extend(cl)

        def drain(n):
            for _ in range(n):
                if pieces:
                    pieces.popleft()()

        # P-step indexing: pass p occupies [33p, 33p+32]; 32 pv + 1 fin batch.
        # op-C (qcp1 g0 partials) woven into pass 3; op0 (qcp0, full octiles)
        # into pass 4; op-D (qcp1 g1 + adds) at the tail.
        opC = list(make_op_g0(1, [(pp_m, "m")]))
        opC_at = {67 + 2 * j: cl for j, cl in enumerate(opC)}
        op0 = list(make_outproj(0, [(pp_m, "m")]))
        op0_at = {100 + j: cl for j, cl in enumerate(op0)}

        si = 2
        pv_done = 0
        for pi, (kind, p) in enumerate(P_all):
            p()
            log(f"P{pi}:{kind}")
            if kind == "pv":
                pv_done += 1
            if pi in opC_at:
                opC_at[pi]()
                log(f"opC@{pi}")
            if pi in op0_at:
                op0_at[pi]()
                log(f"op0@{pi}")
            if kind == "pv":
                drain(2 if pi < 4 else 1)
                log(f"drain@{pi}")
            while si < len(S_all) and si < pv_done + 5:
                S_all[si]()
                log(f"S{si}")
                si += 1
        while si < len(S_all):
            S_all[si]()
            si += 1
        drain(len(pieces))
        log("tail-start")

        # tail: per-lt, PE-transpose the last pass's cn slice into ctxT,
        # immediately followed by that lt's op-D pieces.
        cn_last = _CACHE.pop("last_cn")
        opD = list(make_op_g1(1, [(pp_s, "s"), (pp_s, "s"), (pp_ctx, "ctx")]))
        for lt in range(8):
            tp = pp_m.tile([128, 128], f16, tag="m", name="tpd")
            nc.tensor.transpose(
                tp[:], cn_last[:, lt * 128:(lt + 1) * 128], idn[:])
            nc.vector.tensor_copy(ctxT[1][1][:, lt * 128:(lt + 1) * 128], tp[:])
            opD[2 * lt]()
            opD[2 * lt + 1]()

    nc.compile()
    return nc


def kernel(query, key, values, W1, b1):
    from concourse.bass_utils import run_bass_kernel_spmd

    if "nc" not in _CACHE:
        _CACHE["nc"] = _gen_kernel()
    nc = _CACHE["nc"]

    query = np.asarray(query, dtype=np.float32)
    key = np.asarray(key, dtype=np.float32)
    values = np.asarray(values, dtype=np.float32)
    W1 = np.asarray(W1, dtype=np.float32)
    b1 = np.asarray(b1, dtype=np.float32)

    xT = {}
    for b in range(B):
        xT[("q", b)] = np.ascontiguousarray(query[b].T).astype(np.float16)
        xT[("k", b)] = np.ascontiguousarray(key[b].T).astype(np.float16)
        xT[("v", b)] = np.ascontiguousarray(values[b].T).astype(np.float16)

    in_maps = []
    for core in range(N_CORES):
        b = core // HPC
        hg = core % HPC
        sl = slice(hg * ES, (hg + 1) * ES)
        in_maps.append({
            "xqT": xT[("q", b)],
            "xkT": xT[("k", b)],
            "xvT": xT[("v", b)],
            "wT": np.ascontiguousarray(W1[sl, :].T).astype(np.float16),
            "woT": np.ascontiguousarray(W1[:, sl].T).astype(np.float16),
            "bqkv": np.ascontiguousarray(b1[sl].reshape(2, 128).T),
        })

    res = run_bass_kernel_spmd(
        nc, in_maps, core_ids=list(range(N_CORES)),
        trace=bool(_CACHE.get("trace", False)))
    _CACHE["last_results"] = res

    output = np.empty((B, L, EMBED), dtype=np.float32)
    for b in range(B):
        acc = res.results[b * HPC]["out"].astype(np.float32)
        acc[L // 2:] += res.results[b * HPC]["out2"].astype(np.float32)
        for hg in range(1, HPC):
            acc += res.results[b * HPC + hg]["out"].astype(np.float32)
            acc[L // 2:] += res.results[b * HPC + hg]["out2"].astype(np.float32)
        output[b] = acc + b1[None, :]
    return output


# revision 40
# speedup vs baseline: 1.0217x; 1.0138x over previous
"""Multi-headed attention (B=2, L=2048, E=1024, H=16) on 8 trn2 cores.

Sharding: batch (2) x head-groups (4) -> 8 cores. Each core computes 4 heads
of one batch element end-to-end (QKV projection, attention, partial output
projection); host sums the per-head-group partial outputs (out + out2) per
batch and adds the final bias.

All data is fp16 (host converts); PSUM accumulates fp32. Design, driven by
the TimelineSim cost model (matmul cost = out free size; exp is ACT-only):
  - QKV projections: W-slice stationary, x^T moving (N=512 chains).
  - Scores S^T [k, q] per head into [128,1024] PSUM, one wide exp per
    (head, k-tile) on ACT straight to fp16 SBUF (1038ns each; ACT busy
    ~134us is the co-critical path with PE ~140us).
  - PV flipped to out[q, d]: pt (exp'd scores) stationary, V [k, 64+1]
    moving (N=65; ones column accumulates the softmax denominator) -- half
    the PE rows of the N=q orientation.
  - Normalization fused into the ctx eviction (reciprocal + 0-stride
    broadcast multiply, batched per PSUM bank); ctx^T via DMA transpose
    (mid-kernel) or PE transpose (final pass, avoids DMA latency).
  - Attention runs in 4 passes of 2 heads ordered (qcp0,g0),(qcp1,g0),
    (qcp0,g1),(qcp1,g1) so passes 1-2 reuse g0 K/V and all g1 projection
    chains spread into later passes, balancing PE load per pass.
  - PSUM (8 banks): scores 2x[128,1024] (4) + ctx regions 130-wide pairs
    (3) + misc (1); one start/stop per bank per pass (zero-region rule).
  - Emission is software-pipelined: scores/exp run ~2 chunks ahead of PV;
    projection chains weave in as ~3-matmul micro-pieces; out-projection:
    qcp0 whole octiles in pass 4, qcp1 g0-part DMA'd to out2 during pass 3
    (host adds), g1-part at the tail woven with per-lt PE transposes.
"""

import numpy as np

EMBED = 1024
HEADS = 16
HD = 64
B = 2
L = 2048
N_CORES = 8
HPC = 4              # heads per core
ES = HPC * HD        # 256: e-slice width per core
NEC = EMBED // 128   # 8 embed chunks
NKT = L // 128       # 16 k-tiles

_CACHE = {}
EMITLOG = []


def _gen_kernel():
    from contextlib import ExitStack

    import concourse.mybir as mybir
    import concourse.tile as tile
    from concourse import bacc
    from concourse.masks import make_identity

    dt = mybir.dt
    f32 = dt.float32
    f16 = dt.float16

    nc = bacc.Bacc("TRN2", target_bir_lowering=False)

    xqT = nc.dram_tensor("xqT", [EMBED, L], f16, kind="ExternalInput")
    xkT = nc.dram_tensor("xkT", [EMBED, L], f16, kind="ExternalInput")
    xvT = nc.dram_tensor("xvT", [EMBED, L], f16, kind="ExternalInput")
    wT = nc.dram_tensor("wT", [EMBED, ES], f16, kind="ExternalInput")
    woT = nc.dram_tensor("woT", [ES, EMBED], f16, kind="ExternalInput")
    bqkv = nc.dram_tensor("bqkv", [128, 2], f32, kind="ExternalInput")
    out = nc.dram_tensor("out", [L, EMBED], f16, kind="ExternalOutput")
    out2 = nc.dram_tensor("out2", [L // 2, EMBED], f16, kind="ExternalOutput")

    # ctx region offset: (q8, hh) -> free offset in the 3-bank ctx tile.
    # Pairs of 130 (two heads x 65), 3 pairs per 512-f32 bank, no straddle.
    def ctx_off(q8, hh):
        return (q8 // 3) * 512 + (q8 % 3) * 130 + hh * 65

    with tile.TileContext(nc) as tc, ExitStack() as ctx:
        const = ctx.enter_context(tc.tile_pool(name="const", bufs=1))
        xin = ctx.enter_context(tc.tile_pool(name="xin", bufs=1))
        big = ctx.enter_context(tc.tile_pool(name="big", bufs=1))
        work = ctx.enter_context(tc.tile_pool(name="work", bufs=2))
        # PSUM budget (8 banks): scores 2x2 + ctx 3 + misc 1
        pp_s = ctx.enter_context(tc.tile_pool(name="pp_s", bufs=2, space="PSUM"))
        pp_ctx = ctx.enter_context(tc.tile_pool(name="pp_ctx", bufs=1, space="PSUM"))
        pp_m = ctx.enter_context(tc.tile_pool(name="pp_m", bufs=1, space="PSUM"))

        # ---- constants (g0 half of wt first; wo deferred) ------------------
        wt = const.tile([128, NEC * ES], f16)
        nc.sync.dma_start(
            wt[:].rearrange("p (c e) -> p c e", c=NEC),
            wT[:].rearrange("(c p) e -> p c e", p=128))
        bq = const.tile([128, 2], f32)
        nc.sync.dma_start(bq[:], bqkv[:])

        idn = const.tile([128, 128], f16)
        make_identity(nc, idn[:])

        # ---- x staging (fp16; xq chunk-tiles, xk/xv single tiles) ----------
        xq_sb = xin.tile([128, NEC * L], f16, name="xq")
        xk_sb = xin.tile([128, NEC * L], f16, name="xk")
        xv_sb = xin.tile([128, NEC * L], f16, name="xv")

        def xq_ap(c, qc):
            return xq_sb[:, c * L + qc * 512:(c * L) + (qc + 1) * 512]

        def xk_ap(c, qc):
            return xk_sb[:, c * L + qc * 512:(c * L) + (qc + 1) * 512]

        def xv_ap(c, qc):
            return xv_sb[:, c * L + qc * 512:(c * L) + (qc + 1) * 512]

        def big_dma(xsb, xdram, lo, hi):
            nc.sync.dma_start(
                xsb[:].rearrange("p (c l) -> p c l", l=L)[:, :, lo:hi],
                xdram[:].rearrange("(c p) l -> p c l", p=128)[:, :, lo:hi])

        # startup-critical order: xk-q0 lands between the xq quarters so
        # k00 overlaps q10; q10 is only needed by S0's second matmul
        big_dma(xq_sb, xqT, 0, 512)
        big_dma(xk_sb, xkT, 0, 512)
        big_dma(xq_sb, xqT, 512, 1024)
        big_dma(xv_sb, xvT, 0, 512)
        big_dma(xk_sb, xkT, 512, 1024)
        big_dma(xv_sb, xvT, 512, 1024)
        big_dma(xk_sb, xkT, 1024, 2048)
        big_dma(xv_sb, xvT, 1024, 2048)

        wo = const.tile([128, 2 * EMBED], f16)
        nc.sync.dma_start(
            wo[:].rearrange("p (g e) -> p g e", g=2),
            woT[:].rearrange("(g p) e -> p g e", p=128),
        )

        big_dma(xq_sb, xqT, 1024, 2048)

        # ---- persistent activations ---------------------------------------
        # qt[qcp]: [e-group g partitions (2 heads x 64d), free g*1024 + q]
        qt = [big.tile([128, 2048], f16, name=f"qt{i}") for i in range(2)]
        # ktt[qc]: [d partitions, g*512 + k-local]
        ktt = [big.tile([128, 1024], f16, name=f"ktt{i}") for i in range(4)]
        # vaug[kt]: [k partitions, 4h x (64d + ones)]
        vaug = [big.tile([128, 4 * 65], f16, name=f"vaug{i}") for i in range(NKT)]
        # ctxT[qcp][g]: [c partitions (2 heads x 64d), q 1024]
        ctxT = [[big.tile([128, 1024], f16, name=f"ctxT{i}{g}") for g in range(2)]
                for i in range(2)]

        # ones columns of vaug (denominator trick), one strided memset per kt
        for kt in range(NKT):
            nc.gpsimd.memset(
                vaug[kt][:].rearrange("p (h x) -> p x h", x=65)[:, 64:65, :], 1.0)

        # ---- PE warmup during the DMA-bound prologue -----------------------
        warm = pp_m.tile([128, 512], f32, tag="m")
        for i in range(24):
            nc.tensor.matmul(
                warm[:, 0:128], lhsT=idn[:], rhs=idn[:],
                start=(i == 0), stop=(i == 23))

        # ---- projection chains (as micro-piece closures) --------------------
        def chain_pieces(xap, qc, g, write):
            """Returns piece closures: [3 mms], [3 mms], [2 mms + evict]."""
            state = {}

            def mms(c0, c1):
                def _p():
                    if c0 == 0:
                        state["ps"] = pp_m.tile([128, 512], f32, tag="m",
                                                name="ps")
                    ps = state["ps"]
                    for c in range(c0, c1):
                        nc.tensor.matmul(
                            ps[:],
                            lhsT=wt[:, c * ES + g * 128: c * ES + (g + 1) * 128],
                            rhs=xap(c, qc),
                            start=(c == 0), stop=(c == NEC - 1))
                    if c1 == NEC:
                        write(state["ps"])
                return _p

            return [mms(0, 3), mms(3, 6), mms(6, 8)]

        def chain_q(qc, g):
            def write(ps):
                nc.vector.tensor_scalar_add(
                    qt[qc // 2][:, g * 1024 + (qc % 2) * 512:
                                g * 1024 + (qc % 2 + 1) * 512],
                    ps[:], bq[:, g:g + 1])
            return chain_pieces(xq_ap, qc, g, write)

        def chain_k(qc, g):
            def write(ps):
                nc.vector.tensor_scalar_add(
                    ktt[qc][:, g * 512:(g + 1) * 512], ps[:], bq[:, g:g + 1])
            return chain_pieces(xk_ap, qc, g, write)

        def chain_v(qc, g):
            """k/q pieces plus V transpose pieces into vaug."""
            state = {}

            def write(ps):
                vtt = work.tile([128, 512], f16, tag="vtt", bufs=2)
                nc.vector.tensor_scalar_add(vtt[:], ps[:], bq[:, g:g + 1])
                state["vtt"] = vtt

            pieces = chain_pieces(xv_ap, qc, g, write)

            def tp_piece(j0):
                def _p():
                    vtt = state["vtt"]
                    for j in (j0, j0 + 1):
                        tp = pp_m.tile([128, 128], f16, tag="m", name="tp")
                        nc.tensor.transpose(
                            tp[:], vtt[:, j * 128:(j + 1) * 128], idn[:])
                        nc.vector.tensor_copy(
                            vaug[qc * 4 + j][:, (2 * g) * 65:(2 * g + 2) * 65]
                            .rearrange("p (h x) -> p h x", h=2)[:, :, 0:64],
                            tp[:].rearrange("p (h x) -> p h x", h=2))
                return _p

            return pieces + [tp_piece(0), tp_piece(2)]

        # ---- attention pass: one e-group (2 heads) over one qcp ------------
        inv_sqrt_e = 1.0 / 32.0

        def make_pass(qcp, g, last=False):
            """Returns (S, P): scores/exp chunks and pv/finalize chunks."""
            state = {}

            def scores_chunk(kt, hh):
                sps = pp_s.tile([128, 1024], f32, tag="s")
                off = hh * 64
                for half in range(2):
                    nc.tensor.matmul(
                        sps[:, half * 512:(half + 1) * 512],
                        lhsT=ktt[kt // 4][
                            off:off + 64,
                            g * 512 + (kt % 4) * 128: g * 512 + (kt % 4 + 1) * 128],
                        rhs=qt[qcp][off:off + 64,
                                    g * 1024 + half * 512: g * 1024 + (half + 1) * 512],
                        start=True, stop=True)
                pt = work.tile([128, 1024], f16, tag="pt", bufs=6)
                nc.scalar.activation(
                    pt[:], sps[:], mybir.ActivationFunctionType.Exp,
                    scale=inv_sqrt_e)
                state[("pt", kt, hh)] = pt

            def pv_chunk(kt, hh):
                # one PSUM start/stop per bank per pass (zero-region rule):
                # bank b of ctx starts at (kt0, hh0, q8=3b), stops at
                # (kt15, hh1, q8 = 2/5/7).
                if kt == 0 and hh == 0:
                    state["ctx"] = pp_ctx.tile(
                        [128, 1536], f32, tag="ctx", name="ctxp")
                ctxp = state["ctx"]
                pt = state.pop(("pt", kt, hh))
                h = 2 * g + hh
                for q8 in range(8):
                    o = ctx_off(q8, hh)
                    nc.tensor.matmul(
                        ctxp[:, o: o + 65],
                        lhsT=pt[:, q8 * 128:(q8 + 1) * 128],
                        rhs=vaug[kt][:, h * 65:(h + 1) * 65],
                        start=(kt == 0 and hh == 0 and q8 % 3 == 0),
                        stop=(kt == NKT - 1 and hh == 1 and q8 in (2, 5, 7)))

            def fin_batch():
                """Batched finalize: 3 per-bank recips + 3 per-bank muls
                (rec broadcast via 0-stride), then 8 async DMA transposes
                into ctxT. Short critical chain at the pass boundary."""
                ctxp = state["ctx"]
                rec = work.tile([128, 16], f32, tag="rec", bufs=2, name="rec")
                cn = work.tile([128, 1024], f16, tag="cn", bufs=2, name="cn")
                for b in range(3):
                    npair = 3 if b < 2 else 2
                    den_view = ctxp[:, b * 512: b * 512 + npair * 130].rearrange(
                        "p (r h x) -> p r h x", h=2, x=65)[:, :, :, 64:65]
                    nc.vector.reciprocal(
                        rec[:, b * 6: b * 6 + npair * 2].rearrange(
                            "p (r h x) -> p r h x", h=2, x=1),
                        den_view)
                for b in range(3):
                    npair = 3 if b < 2 else 2
                    nc.vector.tensor_mul(
                        cn[:, b * 3 * 128: (b * 3 + npair) * 128].rearrange(
                            "p (r h x) -> p r h x", h=2, x=64),
                        ctxp[:, b * 512: b * 512 + npair * 130].rearrange(
                            "p (r h x) -> p r h x", h=2, x=65)[:, :, :, 0:64],
                        rec[:, b * 6: b * 6 + npair * 2].rearrange(
                            "p (r h x) -> p r h x", h=2, x=1).broadcast_to(
                            [128, npair, 2, 64]))
                if last:
                    _CACHE["last_cn"] = cn
                    return
                for q8 in range(8):
                    nc.sync.dma_start_transpose(
                        ctxT[qcp][g][:, q8 * 128:(q8 + 1) * 128],
                        cn[:, q8 * 128:(q8 + 1) * 128])

            S = [(lambda kt=kt, hh=hh: scores_chunk(kt, hh))
                 for kt in range(NKT) for hh in range(2)]
            P = [("pv", (lambda kt=kt, hh=hh: pv_chunk(kt, hh)))
                 for kt in range(NKT) for hh in range(2)]
            P += [("fin", fin_batch)]
            return S, P

        # ---- output projection ----------------------------------------------
        def make_outproj(qcp, pool_tags, evict_split=False):
            """Yields single-matmul pieces: (g0 mm) then (g1 mm + evict)."""
            state = {}

            def part(lt, oc, g, slot_i):
                if g == 0:
                    pool, tag = pool_tags[slot_i % len(pool_tags)]
                    state["ops"] = pool.tile([128, 512], f32, tag=tag,
                                             name="ops")
                ops = state["ops"]
                nc.tensor.matmul(
                    ops[:],
                    lhsT=ctxT[qcp][g][:, lt * 128:(lt + 1) * 128],
                    rhs=wo[:, g * EMBED + oc * 512: g * EMBED + (oc + 1) * 512],
                    start=(g == 0), stop=(g == 1))
                if g == 1:
                    if oc == 0:
                        state["ot"] = work.tile([128, 1024], f16, tag="ot",
                                                bufs=4, name="ot")
                    ot = state["ot"]
                    if evict_split and slot_i % 2 == 1:
                        nc.scalar.copy(ot[:, oc * 512:(oc + 1) * 512], ops[:])
                    else:
                        nc.vector.tensor_copy(
                            ot[:, oc * 512:(oc + 1) * 512], ops[:])
                    if oc == 1:
                        lt_g = qcp * 8 + lt
                        nc.sync.dma_start(
                            out[lt_g * 128:(lt_g + 1) * 128, :], ot[:])

            i = 0
            for lt in range(8):
                for oc in range(2):
                    for g in range(2):
                        yield (lambda lt=lt, oc=oc, g=g, i=i: part(lt, oc, g, i))
                    i += 1

        # qcp1 g-split: g0 partials evicted to ot0 staging during pass 3,
        # g1 matmuls + adds + DMA at the tail.
        ot0 = [work.tile([128, 1024], f16, tag="ot0", bufs=8, name=f"ot0_{lt}")
               for lt in range(8)]

        def make_op_g0(qcp, pool_tags):
            def piece(lt, oc, slot_i):
                pool, tag = pool_tags[slot_i % len(pool_tags)]
                ops = pool.tile([128, 512], f32, tag=tag, name="ops0")
                nc.tensor.matmul(
                    ops[:],
                    lhsT=ctxT[qcp][0][:, lt * 128:(lt + 1) * 128],
                    rhs=wo[:, oc * 512:(oc + 1) * 512],
                    start=True, stop=True)
                nc.vector.tensor_copy(ot0[lt][:, oc * 512:(oc + 1) * 512],
                                      ops[:])
                if oc == 1:
                    nc.sync.dma_start(out2[lt * 128:(lt + 1) * 128, :],
                                      ot0[lt][:])

            i = 0
            for lt in range(8):
                for oc in range(2):
                    yield (lambda lt=lt, oc=oc, i=i: piece(lt, oc, i))
                    i += 1

        def make_op_g1(qcp, pool_tags):
            state = {}

            def piece(lt, oc, slot_i):
                pool, tag = pool_tags[slot_i % len(pool_tags)]
                ops = pool.tile([128, 512], f32, tag=tag, name="ops1")
                nc.tensor.matmul(
                    ops[:],
                    lhsT=ctxT[qcp][1][:, lt * 128:(lt + 1) * 128],
                    rhs=wo[:, EMBED + oc * 512: EMBED + (oc + 1) * 512],
                    start=True, stop=True)
                if oc == 0:
                    state["ot"] = work.tile([128, 1024], f16, tag="ot",
                                            bufs=4, name="ot")
                ot = state["ot"]
                if slot_i % 2 == 1:
                    nc.scalar.copy(ot[:, oc * 512:(oc + 1) * 512], ops[:])
                else:
                    nc.vector.tensor_copy(ot[:, oc * 512:(oc + 1) * 512],
                                          ops[:])
                if oc == 1:
                    lt_g = qcp * 8 + lt
                    nc.sync.dma_start(
                        out[lt_g * 128:(lt_g + 1) * 128, :], ot[:])

            i = 0
            for lt in range(8):
                for oc in range(2):
                    yield (lambda lt=lt, oc=oc, i=i: piece(lt, oc, i))
                    i += 1

        # ---- emission schedule ----------------------------------------------
        # Pass order (0,g0),(1,g0),(0,g1),(1,g1): passes 1-2 share the g0
        # K/V tensors so all g1 projection chains defer to later passes,
        # balancing PE load against the ACT-bound exp stream in every pass.
        # interleave k00/q00 chain pieces so both track their DMAs; q00/q10
        # use the (still free) scores psum slots to run parallel to k00
        def chain_q_slot(qc):
            ps = pp_s.tile([128, 512], f32, tag="s", name="psq")
            for c in range(NEC):
                nc.tensor.matmul(
                    ps[:],
                    lhsT=wt[:, c * ES: c * ES + 128],
                    rhs=xq_ap(c, qc),
                    start=(c == 0), stop=(c == NEC - 1))
            nc.vector.tensor_scalar_add(
                qt[0][:, qc * 512:(qc + 1) * 512], ps[:], bq[:, 0:1])

        chain_q_slot(0)
        for piece in chain_k(0, 0):
            piece()
        chain_q_slot(1)


        S_all, P_all = [], []
        for (qcp, g) in [(0, 0), (1, 0), (0, 1), (1, 1)]:
            S, P = make_pass(qcp, g, last=(qcp == 1 and g == 1))
            S_all += S
            P_all += P

        def log(label):
            nm = nc.get_next_instruction_name()  # peeks+consumes one id
            EMITLOG.append((label, int(nm.split("-")[1])))

        S_all[0]()
        log("S0")
        S_all[1]()
        log("S1")
        for piece in chain_v(0, 0):
            piece()
        log("v00")

        from collections import deque
        pieces = deque()
        for cl in (chain_k(1, 0), chain_v(1, 0),
                   chain_k(2, 0), chain_k(3, 0), chain_v(2, 0), chain_v(3, 0),
                   chain_q(2, 0), chain_q(3, 0),
                   chain_k(0, 1), chain_k(1, 1), chain_k(2, 1), chain_k(3, 1),
                   chain_v(0, 1), chain_v(1, 1),
                   chain_q(0, 1), chain_q(1, 1),
                   chain_v(2, 1), chain_v(3, 1),
                   chain_q(2, 1), chain_q(3, 1)):
            pieces.extend(cl)

        def drain(n):
            for _ in range(n):
                if pieces:
                    pieces.popleft()()

        # P-step indexing: pass p occupies [33p, 33p+32]; 32 pv + 1 fin batch.
        # op-C (qcp1 g0 partials) woven into pass 3; op0 (qcp0, full octiles)
        # into pass 4; op-D (qcp1 g1 + adds) at the tail.
        opC = list(make_op_g0(1, [(pp_m, "m")]))
        opC_at = {67 + 2 * j: cl for j, cl in enumerate(opC)}
        op0 = list(make_outproj(0, [(pp_m, "m")]))
        op0_at = {100 + j: cl for j, cl in enumerate(op0)}

        si = 2
        pv_done = 0
        for pi, (kind, p) in enumerate(P_all):
            p()
            log(f"P{pi}:{kind}")
            if kind == "pv":
                pv_done += 1
            if pi in opC_at:
                opC_at[pi]()
                log(f"opC@{pi}")
            if pi in op0_at:
                op0_at[pi]()
                log(f"op0@{pi}")
            if kind == "pv":
                drain(2 if pi < 4 else 1)
                log(f"drain@{pi}")
            while si < len(S_all) and si < pv_done + 5:
                S_all[si]()
                log(f"S{si}")
                si += 1
        while si < len(S_all):
            S_all[si]()
            si += 1
        drain(len(pieces))
        log("tail-start")

        # tail: per-lt, PE-transpose the last pass's cn slice into ctxT,
        # immediately followed by that lt's op-D pieces.
        cn_last = _CACHE.pop("last_cn")
        opD = list(make_op_g1(1, [(pp_s, "s"), (pp_s, "s"), (pp_ctx, "ctx")]))

        def tp_lt(lt):
            tp = pp_m.tile([128, 128], f16, tag="m", name="tpd")
            nc.tensor.transpose(
                tp[:], cn_last[:, lt * 128:(lt + 1) * 128], idn[:])
            nc.vector.tensor_copy(ctxT[1][1][:, lt * 128:(lt + 1) * 128], tp[:])

        # prefetch transposes one lt ahead so the DVE copy overlaps matmuls
        tp_lt(0)
        for lt in range(8):
            if lt + 1 < 8:
                tp_lt(lt + 1)
            opD[2 * lt]()
            opD[2 * lt + 1]()

    nc.compile()
    return nc


def kernel(query, key, values, W1, b1):
    from concourse.bass_utils import run_bass_kernel_spmd

    if "nc" not in _CACHE:
        _CACHE["nc"] = _gen_kernel()
    nc = _CACHE["nc"]

    query = np.asarray(query, dtype=np.float32)
    key = np.asarray(key, dtype=np.float32)
    values = np.asarray(values, dtype=np.float32)
    W1 = np.asarray(W1, dtype=np.float32)
    b1 = np.asarray(b1, dtype=np.float32)

    xT = {}
    for b in range(B):
        xT[("q", b)] = np.ascontiguousarray(query[b].T).astype(np.float16)
        xT[("k", b)] = np.ascontiguousarray(key[b].T).astype(np.float16)
        xT[("v", b)] = np.ascontiguousarray(values[b].T).astype(np.float16)

    in_maps = []
    for core in range(N_CORES):
        b = core // HPC
        hg = core % HPC
        sl = slice(hg * ES, (hg + 1) * ES)
        in_maps.append({
            "xqT": xT[("q", b)],
            "xkT": xT[("k", b)],
            "xvT": xT[("v", b)],
            "wT": np.ascontiguousarray(W1[sl, :].T).astype(np.float16),
            "woT": np.ascontiguousarray(W1[:, sl].T).astype(np.float16),
            "bqkv": np.ascontiguousarray(b1[sl].reshape(2, 128).T),
        })

    res = run_bass_kernel_spmd(
        nc, in_maps, core_ids=list(range(N_CORES)),
        trace=bool(_CACHE.get("trace", False)))
    _CACHE["last_results"] = res

    output = np.empty((B, L, EMBED), dtype=np.float32)
    for b in range(B):
        acc = res.results[b * HPC]["out"].astype(np.float32)
        acc[L // 2:] += res.results[b * HPC]["out2"].astype(np.float32)
        for hg in range(1, HPC):
            acc += res.results[b * HPC + hg]["out"].astype(np.float32)
            acc[L // 2:] += res.results[b * HPC + hg]["out2"].astype(np.float32)
        output[b] = acc + b1[None, :]
    return output
